# revision 1
# baseline (speedup 1.0000x reference)
"""Bass/Trainium2 kernel for BiGraphContrastLayer (GNN message passing).

Computes, for two edge lists (pos/neg) over the same node features:
    h_g = PReLU( D_in^-1/2 A_g D_out^-1/2 feats @ W + b )
returning stack([h_pos, h_neg]) of shape [2, N, Dout].

Strategy (8 NeuronCores, SPMD, no collectives), using the linearity
   (D_in^-1/2 A D_out^-1/2 feats) @ W = D_in^-1/2 A (D_out^-1/2 feats W):

  Phase 1 (y-phase): y_g = bf16( ns_g * (feats @ W) ), computed from a
    host-transposed feats (so featsT tiles are directly the matmul lhsT),
    with the per-node ns scale applied on the PSUM->SBUF read. Stored to
    DRAM per graph.
  Phase 2 (gather/aggregate): edges are bucketed by 128-node dst tile and
    sorted by src. dma_gather (int16, 4 row-banks of <=25088 rows) pulls
    y[src] rows for ~128-edge chunks; a one-hot matmul (lhsT = onehot of
    dst offsets) segment-sums each chunk into the dst tile's PSUM
    accumulator. Final nd-scale + PReLU on DVE, store.

  Host does integer index work only: degree bincounts, sorting, bucketing,
  dealing dst tiles to cores so all 8 cores share one instruction stream
  (signature-matched by per-bank chunk counts), building int16 wrapped
  gather indices, and replicating the small params per the sharding hint.
"""

import math
import tempfile
from dataclasses import dataclass

import numpy as np

P = 128   # partitions
D = 128   # feature dim (Din == Dout == 128)
NBANK = 4


# --------------------------------------------------------------------------
# Config
# --------------------------------------------------------------------------
@dataclass
class Config:
    n_nodes: int = 100000
    n_cores: int = 8
    xbatch: int = 8    # feats tiles per y-phase batch
    sg: int = 10       # dst-tile positions per gather supergroup
    y_act_split: bool = True  # pos-graph y scale on ScalarE, neg on DVE
    act_prelu: bool = True    # final nd-scale+PReLU on ScalarE (not in sim)
    oh_gpsimd_mod: int = 0    # every Nth one-hot build goes to GpSimd (0=off)
    gbufs: int = 2            # gather buffer count

    @property
    def t_global(self) -> int:
        return math.ceil(self.n_nodes / P)

    @property
    def n_pad(self) -> int:
        return self.t_global * P

    @property
    def t_core(self) -> int:
        return math.ceil(self.t_global / self.n_cores)

    @property
    def bank_tiles(self) -> int:
        return math.ceil(self.t_global / NBANK)

    @property
    def bank_rows(self) -> int:
        return self.bank_tiles * P


# --------------------------------------------------------------------------
# Host-side preprocessing (integer index manipulation only)
# --------------------------------------------------------------------------
def _row_of(n, cfg: Config):
    """y DRAM row of node n: within each xbatch of tiles, rows are laid
    p-major (node (t0+i)*128+p -> row t0*128 + p*nb + i) so the y-phase
    store writes nb*256B contiguous per partition."""
    xb, tg = cfg.xbatch, cfg.t_global
    t = n // P
    p = n % P
    t0 = (t // xb) * xb
    nb = np.minimum(xb, tg - t0)
    return t0 * P + p * nb + (t - t0)


def _plan_graph(src, dst, cfg: Config):
    """Bucket edges by dst tile, sort by src within tile, bank-split, and
    deal tiles to cores with per-bank-signature matching.

    Returns dict with:
      core_tiles  [n_cores, t_core]  global tile id per position (-1 null)
      cstar       [t_core, NBANK]    shared chunk counts per position/bank
      tile_edges  list per global tile: (src_sorted, off_sorted, bank_cnt)
    """
    tg, ncores, tcore = cfg.t_global, cfg.n_cores, cfg.t_core
    order = np.argsort(dst, kind="stable")
    src_s = src[order]
    dst_s = dst[order]
    tile_cnt = np.bincount(dst_s // P, minlength=tg)
    starts = np.zeros(tg + 1, np.int64)
    np.cumsum(tile_cnt, out=starts[1:])

    row_s = _row_of(src_s, cfg)
    bank_of = row_s // cfg.bank_rows
    sig = np.zeros((tg, NBANK), np.int64)
    tile_edges = []
    for t in range(tg):
        e0, e1 = int(starts[t]), int(starts[t + 1])
        so = np.argsort(row_s[e0:e1], kind="stable")
        ts_src = row_s[e0:e1][so]
        ts_off = (dst_s[e0:e1][so] % P).astype(np.int64)
        bc = np.bincount(bank_of[e0:e1], minlength=NBANK)
        sig[t] = -(-bc // P)  # ceil chunks per bank
        sig[t, 0] = max(sig[t, 0], 1)
        tile_edges.append((ts_src, ts_off, bc))

    # Deal: sort tiles by signature so consecutive groups of n_cores tiles
    # have matching/near-matching per-bank chunk counts.
    sigkey = sig @ (np.int64(32) ** np.arange(NBANK - 1, -1, -1))
    keys = np.argsort(sigkey, kind="stable")
    # pad with null tiles to n_cores * t_core
    n_slots = ncores * tcore
    dealt = np.full(n_slots, -1, np.int64)
    dealt[: len(keys)] = keys[::-1]  # descending signature order
    core_tiles = np.zeros((ncores, tcore), np.int64)
    cstar = np.zeros((tcore, NBANK), np.int64)
    for k in range(tcore):
        grp = dealt[k * ncores : (k + 1) * ncores]
        for c in range(ncores):
            core_tiles[c, k] = grp[c]
        s = np.zeros(NBANK, np.int64)
        for t in grp:
            if t >= 0:
                s = np.maximum(s, sig[t])
        s[0] = max(s[0], 1)
        cstar[k] = s
    return dict(core_tiles=core_tiles, cstar=cstar, tile_edges=tile_edges)


def _slot_layout(cstar, cfg: Config):
    """Shared (all-cores) slot layout for one graph.

    Slot space = sequence of supergroups; within a supergroup, bank-major:
      for b in banks: for k in sg positions: cstar[k, b] chunks.
    Returns:
      sg_list: list of (k0, kn)
      calls:   list of (sg_idx, bank, chunk0, nchunks)  [nchunks > 0]
      chunk_of: dict (k, b, c) -> global chunk index
      n_chunks total
    """
    tcore, sg = cfg.t_core, cfg.sg
    sg_list = []
    k0 = 0
    while k0 < tcore:
        kn = min(sg, tcore - k0)
        sg_list.append((k0, kn))
        k0 += kn
    calls = []
    chunk_of = {}
    cpos = 0
    for si, (k0, kn) in enumerate(sg_list):
        for b in range(NBANK):
            c0 = cpos
            for ki in range(kn):
                k = k0 + ki
                for c in range(int(cstar[k, b])):
                    chunk_of[(k, b, c)] = cpos
                    cpos += 1
            if cpos > c0:
                calls.append((si, b, c0, cpos - c0))
    return sg_list, calls, chunk_of, cpos


def _fill_core_graph(plan, layout, core, cfg: Config):
    """Build IDX16 (wrapped) and OFF arrays for one core, one graph."""
    sg_list, calls, chunk_of, n_chunks = layout
    cstar = plan["cstar"]
    idx = np.zeros((n_chunks, P), np.int16)
    off = np.full((n_chunks, P), 512.0, np.float32)
    for k in range(cfg.t_core):
        t = plan["core_tiles"][core, k]
        if t < 0:
            continue
        ts_src, ts_off, bc = plan["tile_edges"][t]
        bstart = np.zeros(NBANK + 1, np.int64)
        np.cumsum(bc, out=bstart[1:])
        for b in range(NBANK):
            nb = int(bc[b])
            cnum = int(cstar[k, b])
            if cnum == 0:
                continue
            nslot = cnum * P
            es = np.zeros(nslot, np.int64)
            eo = np.full(nslot, 512.0, np.float32)
            es[:nb] = ts_src[bstart[b] : bstart[b + 1]] - b * cfg.bank_rows
            eo[:nb] = ts_off[bstart[b] : bstart[b + 1]]
            for c in range(cnum):
                ci = chunk_of[(k, b, c)]
                idx[ci] = es[c * P : (c + 1) * P]
                off[ci] = eo[c * P : (c + 1) * P]
    # wrap: flat slot i (within a call's range) -> [i%16, i//16], replicated
    # to 128 partitions. Build per call, concatenated along columns.
    ncols = n_chunks * P // 16
    idx_w = np.zeros((P, ncols), np.int16)
    for (si, b, c0, nch) in calls:
        flat = idx[c0 : c0 + nch].reshape(-1)  # ni slots
        blk = flat.reshape(-1, 16).T  # [16, ni/16]
        idx_w[:, c0 * 8 : (c0 + nch) * 8] = np.tile(blk, (8, 1))
    return idx_w, off.T.copy()  # off -> [P, n_chunks] f32


def preprocess(feats, W, b, prelu_a, src_pos, dst_pos, src_neg, dst_neg,
               cfg: Config):
    n, ncores, tcore, tg = cfg.n_nodes, cfg.n_cores, cfg.t_core, cfg.t_global
    feats = np.asarray(feats, np.float32)
    W = np.asarray(W, np.float32)
    b = np.asarray(b, np.float32)
    prelu_a = np.asarray(prelu_a, np.float32)

    feats_pad = np.zeros((cfg.n_pad, D), np.float32)
    feats_pad[:n] = feats
    featsT = np.ascontiguousarray(feats_pad.T)  # [D, n_pad]

    plans, layouts, deg_outs, deg_ins = [], [], [], []
    for src, dst in ((src_pos, dst_pos), (src_neg, dst_neg)):
        src = np.asarray(src, np.int64)
        dst = np.asarray(dst, np.int64)
        deg_outs.append(np.bincount(src, minlength=n).astype(np.int32))
        deg_ins.append(np.bincount(dst, minlength=n).astype(np.int32))
        plan = _plan_graph(src, dst, cfg)
        plans.append(plan)
        layouts.append(_slot_layout(plan["cstar"], cfg))

    dego_arr = np.zeros((P, 2 * tg), np.int32)
    for g in range(2):
        dpad = np.zeros(cfg.n_pad, np.int32)
        dpad[:n] = deg_outs[g]
        dego_arr[:, g * tg : (g + 1) * tg] = dpad.reshape(tg, P).T

    degi_arr = np.zeros((ncores, P, 2 * tcore), np.int32)
    for g in range(2):
        dpad = np.zeros(cfg.n_pad, np.int32)
        dpad[:n] = deg_ins[g]
        dpad_t = dpad.reshape(tg, P).T
        for core in range(ncores):
            ct = plans[g]["core_tiles"][core]
            valid = ct >= 0
            degi_arr[core, :, g * tcore : (g + 1) * tcore][:, valid] = (
                dpad_t[:, ct[valid]])

    iota = np.tile(np.arange(P, dtype=np.float32), (P, 1)).astype(
        np.dtype("bfloat16"))
    a_rep = np.full((P, 1), float(prelu_a.reshape(-1)[0]), np.float32)
    b_rep = np.tile(b.reshape(1, D), (P, 1)).astype(np.float32)

    in_maps = []
    for core in range(ncores):
        iw_p, off_p = _fill_core_graph(plans[0], layouts[0], core, cfg)
        iw_n, off_n = _fill_core_graph(plans[1], layouts[1], core, cfg)
        in_maps.append({
            "featst": featsT,
            "w_in": W,
            "a_rep": a_rep,
            "b_rep": b_rep,
            "dego": dego_arr,
            "degi": degi_arr[core],
            "idx_in": np.concatenate([iw_p, iw_n], axis=1),
            "off_in": np.concatenate([off_p, off_n], axis=1),
            "iota_in": iota,
        })
    meta = {
        "layouts": layouts,
        "cstar": [plans[0]["cstar"], plans[1]["cstar"]],
        "use_bias": bool(np.any(b != 0.0)),
    }
    return in_maps, plans, meta


# --------------------------------------------------------------------------
# Device kernel builder
# --------------------------------------------------------------------------
def build_kernel(nc, tc, cfg: Config, meta):
    from contextlib import ExitStack

    import concourse.mybir as mybir

    f32 = mybir.dt.float32
    bf16 = mybir.dt.bfloat16
    i32 = mybir.dt.int32
    i16 = mybir.dt.int16
    Alu = mybir.AluOpType
    Act = mybir.ActivationFunctionType

    tg, tcore, npad = cfg.t_global, cfg.t_core, cfg.n_pad
    layouts = meta["layouts"]
    cstar = meta["cstar"]
    use_bias = meta["use_bias"]
    n_chunks = [layouts[g][3] for g in range(2)]
    ncols = [n_chunks[g] * P // 16 for g in range(2)]

    featst = nc.dram_tensor("featst", [P, npad], f32, kind="ExternalInput").ap()
    w_in = nc.dram_tensor("w_in", [P, D], f32, kind="ExternalInput").ap()
    a_rep = nc.dram_tensor("a_rep", [P, 1], f32, kind="ExternalInput").ap()
    b_rep = nc.dram_tensor("b_rep", [P, D], f32, kind="ExternalInput").ap()
    dego = nc.dram_tensor("dego", [P, 2 * tg], i32, kind="ExternalInput").ap()
    degi = nc.dram_tensor("degi", [P, 2 * tcore], i32, kind="ExternalInput").ap()
    idx_in = nc.dram_tensor("idx_in", [P, sum(ncols)], i16,
                            kind="ExternalInput").ap()
    off_in = nc.dram_tensor("off_in", [P, sum(n_chunks)], f32,
                            kind="ExternalInput").ap()
    iota_in = nc.dram_tensor("iota_in", [P, P], bf16, kind="ExternalInput").ap()
    out = nc.dram_tensor("out", [2, tcore, P, D], f32, kind="ExternalOutput").ap()

    y_dram = [nc.dram_tensor(f"y{g}", [npad, D], bf16, kind="Internal").ap()
              for g in range(2)]

    with ExitStack() as ctx:
        const = ctx.enter_context(tc.tile_pool(name="const", bufs=1))
        work = ctx.enter_context(tc.tile_pool(name="work", bufs=2))
        xpool = ctx.enter_context(tc.tile_pool(name="xpool", bufs=3))
        mpool = ctx.enter_context(tc.tile_pool(name="mpool", bufs=3))
        gpool = ctx.enter_context(tc.tile_pool(name="gpool", bufs=cfg.gbufs))
        import os as _os
        ipool = ctx.enter_context(tc.tile_pool(
            name="ipool", bufs=int(_os.environ.get("IPB", "3"))))
        ohpool = ctx.enter_context(tc.tile_pool(name="ohpool", bufs=6))
        tpool = ctx.enter_context(tc.tile_pool(name="tpool", bufs=4))
        spool = ctx.enter_context(tc.tile_pool(name="spool", bufs=3))
        ypool = ctx.enter_context(tc.tile_pool(
            name="ypool", bufs=int(_os.environ.get("YPB", "4")), space="PSUM"))
        ppool = ctx.enter_context(tc.tile_pool(
            name="ppool", bufs=int(_os.environ.get("PPB", "4")), space="PSUM"))

        # ---- constants ----
        w_sb = const.tile([P, D], bf16)
        nc.gpsimd.dma_start(out=w_sb[:], in_=w_in)  # f32 -> bf16 cast DMA
        iota_sb = const.tile([P, P], bf16)
        nc.sync.dma_start(out=iota_sb[:], in_=iota_in)
        a_sb = const.tile([P, 1], f32)
        nc.sync.dma_start(out=a_sb[:], in_=a_rep)
        if use_bias:
            b_sb = const.tile([P, D], f32)
            nc.sync.dma_start(out=b_sb[:], in_=b_rep)

        # ---- norms from degrees:  norm = (deg>0) / sqrt(max(deg,1)) ----
        def make_norm(deg_ap, width, tagn):
            dg = work.tile([P, width], i32, tag=f"dg{tagn}")
            nc.sync.dma_start(out=dg[:], in_=deg_ap)
            f = work.tile([P, width], f32, tag=f"f{tagn}")
            nc.vector.tensor_copy(out=f[:], in_=dg[:])
            m = work.tile([P, width], f32, tag=f"m{tagn}")
            nc.vector.tensor_scalar(out=m[:], in0=f[:], scalar1=1.0,
                                    scalar2=None, op0=Alu.max)
            r = work.tile([P, width], f32, tag=f"r{tagn}")
            nc.vector.reciprocal(out=r[:], in_=m[:])
            s = work.tile([P, width], f32, tag=f"s{tagn}")
            nc.scalar.activation(out=s[:], in_=r[:], func=Act.Sqrt)
            z = work.tile([P, width], f32, tag=f"z{tagn}")
            nc.vector.tensor_scalar(out=z[:], in0=f[:], scalar1=1.0,
                                    scalar2=None, op0=Alu.min)
            ns = const.tile([P, width], f32, tag=f"o{tagn}")
            nc.vector.tensor_tensor(out=ns[:], in0=s[:], in1=z[:], op=Alu.mult)
            return ns

        ns_sb = make_norm(dego, 2 * tg, "o")       # out-deg norms, all nodes
        nd_sb = make_norm(degi, 2 * tcore, "i")    # in-deg norms, owned slots
        and_sb = const.tile([P, 2 * tcore], f32)
        nc.vector.tensor_tensor(out=and_sb[:], in0=nd_sb[:],
                                in1=a_sb[:, :1].to_broadcast([P, 2 * tcore]),
                                op=Alu.mult)

        # ---- y-phase: y_g = bf16(ns_g * (feats @ W)) ----
        t0 = 0
        while t0 < tg:
            nb = min(cfg.xbatch, tg - t0)
            ld = xpool.tile([P, nb * P], f32, tag="xload")
            nc.sync.dma_start(out=ld[:], in_=featst[:, t0 * P : (t0 + nb) * P])
            ldb = xpool.tile([P, nb * P], bf16, tag="xcast")
            nc.vector.tensor_copy(out=ldb[:], in_=ld[:])
            ybuf0 = xpool.tile([P, nb, D], bf16, tag="ybuf0")
            ybuf1 = xpool.tile([P, nb, D], bf16, tag="ybuf1")
            ybuf = [ybuf0, ybuf1]
            for i in range(nb):
                psy = ypool.tile([P, D], f32)
                nc.tensor.matmul(out=psy[:], lhsT=ldb[:, i * P : (i + 1) * P],
                                 rhs=w_sb[:], start=True, stop=True)
                col = t0 + i
                if cfg.y_act_split:
                    nc.scalar.activation(out=ybuf[0][:, i, :], in_=psy[:],
                                         func=Act.Copy,
                                         scale=ns_sb[:, col : col + 1])
                else:
                    nc.vector.tensor_scalar(out=ybuf[0][:, i, :], in0=psy[:],
                                            scalar1=ns_sb[:, col : col + 1],
                                            scalar2=None, op0=Alu.mult)
                nc.vector.tensor_scalar(out=ybuf[1][:, i, :], in0=psy[:],
                                        scalar1=ns_sb[:, tg + col : tg + col + 1],
                                        scalar2=None, op0=Alu.mult)
            for g in range(2):
                nc.sync.dma_start(
                    out=y_dram[g][t0 * P : (t0 + nb) * P, :].rearrange(
                        "(p i) d -> p i d", i=nb),
                    in_=ybuf[g][:])
            t0 += nb

        # ---- gather + one-hot segment-sum + nd-scale + prelu ----
        col_base = [0, ncols[0]]          # idx column offset per graph
        chk_base = [0, n_chunks[0]]       # off column offset per graph
        cbs_all = []
        for g in range(2):
            calls_by_sg = {}
            for (si, b, c0, nch) in layouts[g][1]:
                calls_by_sg.setdefault(si, []).append((b, c0, nch))
            cbs_all.append(calls_by_sg)
        # interleave the two graphs' supergroups so one graph's gathers fill
        # DMA while the other's PSUM chain drains
        jobs = []
        for si in range(max(len(layouts[0][0]), len(layouts[1][0]))):
            for g in range(2):
                if si < len(layouts[g][0]):
                    jobs.append((g, si))
        for (g, si) in jobs:
            sg_list, calls, chunk_of, _ = layouts[g]
            cs = cstar[g]
            calls_by_sg = cbs_all[g]
            if True:
                (k0, kn) = sg_list[si]
                sg_chunks = sum(int(cs[k0 + ki, b]) for ki in range(kn)
                                for b in range(NBANK))
                c0_sg = chunk_of[(k0, 0, 0)]
                gt = gpool.tile([P, sg_chunks, D], bf16, tag="gather")
                it = ipool.tile([P, sg_chunks * 8], i16, tag="gidx")
                nc.sync.dma_start(
                    out=it[:],
                    in_=idx_in[:, col_base[g] + c0_sg * 8 :
                               col_base[g] + (c0_sg + sg_chunks) * 8])
                ot = ipool.tile([P, sg_chunks], f32, tag="goff")
                nc.sync.dma_start(
                    out=ot[:],
                    in_=off_in[:, chk_base[g] + c0_sg :
                               chk_base[g] + c0_sg + sg_chunks])
                for (b, c0, nch) in calls_by_sg[si]:
                    lo = c0 - c0_sg
                    bank_rows = min(cfg.bank_rows, npad - b * cfg.bank_rows)
                    nc.gpsimd.dma_gather(
                        out_ap=gt[:, lo : lo + nch, :],
                        in_ap=y_dram[g][b * cfg.bank_rows :
                                        b * cfg.bank_rows + bank_rows, :],
                        idxs_ap=it[:, lo * 8 : (lo + nch) * 8],
                        num_idxs=nch * P, num_idxs_reg=nch * P,
                        elem_size=D, single_packet=False)
                stg = spool.tile([P, kn, D], f32, tag="stg")
                for ki in range(kn):
                    k = k0 + ki
                    nonzero = [(b, c) for b in range(NBANK)
                               for c in range(int(cs[k, b]))]
                    ps_a = ppool.tile([P, D], f32)
                    for j, (b, c) in enumerate(nonzero):
                        ci = chunk_of[(k, b, c)]
                        lo = ci - c0_sg
                        oh = ohpool.tile([P, P], bf16)
                        eng = nc.vector
                        if cfg.oh_gpsimd_mod and (ci % cfg.oh_gpsimd_mod == 0):
                            eng = nc.gpsimd
                        eng.tensor_scalar(
                            out=oh[:], in0=iota_sb[:],
                            scalar1=ot[:, lo : lo + 1],
                            scalar2=None, op0=Alu.is_equal)
                        nc.tensor.matmul(
                            out=ps_a[:], lhsT=oh[:], rhs=gt[:, lo, :],
                            start=(j == 0), stop=(j == len(nonzero) - 1))
                    kslot = g * tcore + k
                    if cfg.act_prelu and not use_bias:
                        nc.scalar.activation(
                            out=stg[:, ki, :], in_=ps_a[:], func=Act.Prelu,
                            scale=nd_sb[:, kslot : kslot + 1],
                            alpha=a_sb[:, :1])
                        continue
                    if use_bias:
                        hb = tpool.tile([P, D], f32, tag="hb")
                        nc.vector.tensor_scalar(
                            out=hb[:], in0=ps_a[:],
                            scalar1=nd_sb[:, kslot : kslot + 1],
                            scalar2=None, op0=Alu.mult)
                        hb2 = tpool.tile([P, D], f32, tag="hb2")
                        nc.vector.tensor_tensor(out=hb2[:], in0=hb[:],
                                                in1=b_sb[:], op=Alu.add)
                        neg = tpool.tile([P, D], f32, tag="neg")
                        nc.vector.tensor_scalar(
                            out=neg[:], in0=hb2[:], scalar1=0.0,
                            scalar2=a_sb[:, :1], op0=Alu.min, op1=Alu.mult)
                        pos = tpool.tile([P, D], f32, tag="pos")
                        nc.vector.tensor_scalar(
                            out=pos[:], in0=hb2[:], scalar1=0.0,
                            scalar2=None, op0=Alu.max)
                    else:
                        neg = tpool.tile([P, D], f32, tag="neg")
                        nc.vector.tensor_scalar(
                            out=neg[:], in0=ps_a[:], scalar1=0.0,
                            scalar2=and_sb[:, kslot : kslot + 1],
                            op0=Alu.min, op1=Alu.mult)
                        pos = tpool.tile([P, D], f32, tag="pos")
                        nc.vector.tensor_scalar(
                            out=pos[:], in0=ps_a[:], scalar1=0.0,
                            scalar2=nd_sb[:, kslot : kslot + 1],
                            op0=Alu.max, op1=Alu.mult)
                    nc.vector.tensor_tensor(out=stg[:, ki, :], in0=neg[:],
                                            in1=pos[:], op=Alu.add)
                nc.sync.dma_start(
                    out=out[g, k0 : k0 + kn, :, :].rearrange("k p d -> p k d"),
                    in_=stg[:])
    return out


# --------------------------------------------------------------------------
# Driver
# --------------------------------------------------------------------------
def _build_program(cfg: Config, meta):
    import concourse.bacc as bacc
    import concourse.tile as tile

    nc = bacc.Bacc("TRN2", target_bir_lowering=False, debug=False,
                   enable_asserts=False, num_devices=cfg.n_cores)
    with tile.TileContext(nc) as tc:
        build_kernel(nc, tc, cfg, meta)
    nc.compile()
    return nc


def _unscramble(results, plans, cfg: Config):
    n = cfg.n_nodes
    full = np.zeros((2, n, D), np.float32)
    for g in range(2):
        ct_all = plans[g]["core_tiles"]
        for core in range(cfg.n_cores):
            oc = results[core]["out"]  # [2, t_core, P, D]
            for k in range(cfg.t_core):
                t = int(ct_all[core, k])
                if t < 0:
                    continue
                r0 = t * P
                r1 = min(r0 + P, n)
                full[g, r0:r1] = oc[g, k, : r1 - r0, :]
    return full


_PROGRAM_CACHE = {}


def run(inputs, cfg: Config, trace=False):
    from concourse.bass_utils import run_bass_kernel_spmd

    in_maps, plans, meta = preprocess(
        inputs["feats"], inputs["W"], inputs["b"], inputs["prelu_a"],
        inputs["src_pos"], inputs["dst_pos"],
        inputs["src_neg"], inputs["dst_neg"], cfg)

    key = (cfg.n_nodes, cfg.n_cores, cfg.xbatch, cfg.sg, cfg.y_act_split,
           cfg.act_prelu, cfg.oh_gpsimd_mod, cfg.gbufs,
           meta["cstar"][0].tobytes(), meta["cstar"][1].tobytes(),
           meta["use_bias"])
    nc = _PROGRAM_CACHE.get(key)
    if nc is None:
        nc = _build_program(cfg, meta)
        _PROGRAM_CACHE[key] = nc

    kwargs = {}
    if trace:
        kwargs = dict(trace=True, tmpdir=tempfile.mkdtemp(prefix="bgc_trace_"))
    res = run_bass_kernel_spmd(nc, in_maps, core_ids=list(range(cfg.n_cores)),
                               **kwargs)
    full = _unscramble(res.results, plans, cfg)
    return full, res


def kernel(**inputs) -> np.ndarray:
    cfg = Config()
    full, _ = run(inputs, cfg)
    return full



# revision 2
# speedup vs baseline: 1.5734x; 1.5734x over previous
"""Bass/Trainium2 kernel for BiGraphContrastLayer (GNN message passing).

Computes, for two edge lists (pos/neg) over the same node features:
    h_g = PReLU( D_in^-1/2 A_g D_out^-1/2 feats @ W + b )
returning stack([h_pos, h_neg]) of shape [2, N, Dout].

Strategy (8 NeuronCores, SPMD, no collectives). Using linearity,
    (D_in^-1/2 A D_out^-1/2 feats) @ W = D_in^-1/2 (A (D_out^-1/2 feats)) @ W
so the device aggregates raw (host-prescaled) feature rows FIRST and applies
W once per destination tile afterwards — there is no feats@W precompute
phase and no intermediate DRAM round trip at all:

  Host: x_g = f16(ns_g * feats)  (per-graph out-degree prescale), edges
  bucketed by dst tile, dst tiles dealt to cores (prefix-balanced so all 8
  cores share one instruction stream), edges packed into 128-slot chunks at
  supergroup x bank granularity (bank = 32K-row window for int16 gather
  indices; padding ~2%).

  Device, per (graph, supergroup) job:
    dma_gather pulls x[src] rows for each bank region into SBUF; per dst
    tile, one-hot matmuls (lhsT = gathered rows, rhs = is_equal(iota, off))
    segment-sum into a TRANSPOSED PSUM accumulator aggT[d, j]; ScalarE
    copies aggT to SBUF f16; one matmul aggT^T @ W -> h[j, d']; ScalarE
    PReLU with the in-degree norm nd folded into the activation scale
    (prelu(s*x) = s*prelu(x) for s>=0). f16 output, host upcasts.
"""

import math
import tempfile
from dataclasses import dataclass

import numpy as np

P = 128   # partitions
D = 128   # feature dim (Din == Dout == 128)
NBANK = 4


# --------------------------------------------------------------------------
# Config
# --------------------------------------------------------------------------
@dataclass
class Config:
    n_nodes: int = 100000
    n_cores: int = 8
    sg: int = 25       # dst-tile positions per supergroup
    oh_mod: int = 6    # every oh_mod-th one-hot build goes to GpSimd (0=off)
    act_prelu: bool = True   # final PReLU on ScalarE (not in CoreSim)
    gbufs: int = 2           # gather buffer count

    @property
    def t_global(self) -> int:
        return math.ceil(self.n_nodes / P)

    @property
    def n_pad(self) -> int:
        return self.t_global * P

    @property
    def t_core(self) -> int:
        return math.ceil(self.t_global / self.n_cores)

    @property
    def bank_tiles(self) -> int:
        return math.ceil(self.t_global / NBANK)

    @property
    def bank_rows(self) -> int:
        return self.bank_tiles * P


# --------------------------------------------------------------------------
# Host-side preprocessing
# --------------------------------------------------------------------------
def _deal_tiles(cnt, cfg: Config):
    """Deal tiles to cores, prefix-balanced: sort tiles by count desc, then
    per group of n_cores assign the largest remaining tile to the core with
    the smallest running total, keeping per-core slot-prefix sums aligned.
    Returns core_tiles [n_cores, t_core] (-1 = null)."""
    tg, nc, tc = cfg.t_global, cfg.n_cores, cfg.t_core
    order = np.argsort(-cnt, kind="stable")
    core_tiles = np.full((nc, tc), -1, np.int64)
    cum = np.zeros(nc, np.int64)
    for k in range(tc):
        grp = order[k * nc: (k + 1) * nc]
        cores = np.argsort(cum, kind="stable")  # smallest cumsum first
        for i, t in enumerate(grp):             # biggest tile first
            c = cores[i]
            core_tiles[c, k] = t
            cum[c] += cnt[t]
    return core_tiles


def _sg_split(tc, sg):
    out = []
    k0 = 0
    while k0 < tc:
        kn = min(sg, tc - k0)
        out.append((k0, kn))
        k0 += kn
    return out


def _layout_graph(src, dst, core_tiles, cfg: Config):
    """Shared chunk/build layout for one graph + per-core idx/off data.

    Returns dict:
      sgs:    list of (k0, kn)
      C:      [n_sg, NBANK] shared chunk counts
      gbase:  [n_sg, NBANK] chunk offset of bank region within the sg tile
      builds: per sg, list of (ki, gcol, col, start, stop); col is global
              off-column index; gcol is chunk index within the sg gather tile
      n_cols: total off columns
      idx:    per-core [P, total_chunks*8] int16 (wrapped gather indices)
      off:    per-core [P, n_cols] f32
      nseg:   total chunks (sum of C)
    """
    ncores, tc = cfg.n_cores, cfg.t_core
    sgs = _sg_split(tc, cfg.sg)
    n_sg = len(sgs)
    brows = cfg.bank_rows

    # per-core, per-tile-position, per-bank edge lists (rows, offs)
    tile_edges = [[None] * tc for _ in range(ncores)]
    order = np.argsort(dst, kind="stable")
    src_s = src[order]
    dst_s = dst[order]
    tstart = np.zeros(cfg.t_global + 1, np.int64)
    np.cumsum(np.bincount(dst_s // P, minlength=cfg.t_global), out=tstart[1:])
    for c in range(ncores):
        for k in range(tc):
            t = core_tiles[c, k]
            if t < 0:
                tile_edges[c][k] = None
                continue
            e0, e1 = int(tstart[t]), int(tstart[t + 1])
            rows = src_s[e0:e1]
            offs = (dst_s[e0:e1] % P).astype(np.int64)
            bank = rows // brows
            bo = np.argsort(bank, kind="stable")
            rows, offs, bank = rows[bo], offs[bo], bank[bo]
            bcut = np.searchsorted(bank, np.arange(NBANK + 1))
            tile_edges[c][k] = (rows, offs, bcut)

    C = np.zeros((n_sg, NBANK), np.int64)
    gbase = np.zeros((n_sg, NBANK), np.int64)
    seg = []  # per sg, per bank: per core: list of (k, s0, s1) slot ranges
    for si, (k0, kn) in enumerate(sgs):
        for b in range(NBANK):
            percore = []
            maxm = 0
            for c in range(ncores):
                pos = 0
                rl = []
                for ki in range(kn):
                    te = tile_edges[c][k0 + ki]
                    if te is None:
                        rl.append((ki, pos, pos))
                        continue
                    n = int(te[2][b + 1] - te[2][b])
                    rl.append((ki, pos, pos + n))
                    pos += n
                percore.append(rl)
                maxm = max(maxm, pos)
            C[si, b] = -(-maxm // 128)
            seg.append(percore)
        C[si, 0] = max(C[si, 0], 1)  # dummy-build anchor
        gbase[si] = np.concatenate([[0], np.cumsum(C[si])[:-1]])

    # shared build list: union of chunk ranges over cores
    builds = []
    n_cols = 0
    for si, (k0, kn) in enumerate(sgs):
        bl = []
        per_k = [[] for _ in range(kn)]
        for b in range(NBANK):
            percore = seg[si * NBANK + b]
            for ki in range(kn):
                lo, hi = None, None
                for c in range(ncores):
                    _, s0, s1 = percore[c][ki]
                    if s1 > s0:
                        l, h = s0 // 128, -(-s1 // 128)
                        lo = l if lo is None else min(lo, l)
                        hi = h if hi is None else max(hi, h)
                if lo is not None:
                    for ch in range(lo, hi):
                        per_k[ki].append((b, ch))
        for ki in range(kn):
            if not per_k[ki]:
                per_k[ki].append((0, 0))  # dummy: zeroes the psum
            nb = len(per_k[ki])
            for j, (b, ch) in enumerate(per_k[ki]):
                bl.append((ki, int(gbase[si, b] + ch), n_cols,
                           j == 0, j == nb - 1))
                n_cols += 1
        builds.append(bl)

    # per-core arrays
    total_chunks = int(C.sum())
    idx_all = np.zeros((ncores, P, total_chunks * 8), np.int16)
    off_all = np.full((ncores, P, n_cols), 512.0, np.float32)
    # chunk column base per (si, b)
    cb = np.concatenate([[0], np.cumsum(C.reshape(-1))[:-1]]).reshape(
        n_sg, NBANK)
    for si, (k0, kn) in enumerate(sgs):
        for b in range(NBANK):
            nslot = int(C[si, b]) * 128
            if nslot == 0:
                continue
            percore = seg[si * NBANK + b]
            for c in range(ncores):
                rows = np.zeros(nslot, np.int64)
                offs = np.full(nslot, 512.0, np.float32)
                ktag = np.full(nslot, -1, np.int64)
                for (ki, s0, s1) in percore[c]:
                    if s1 == s0:
                        continue
                    te = tile_edges[c][k0 + ki]
                    e0, e1 = int(te[2][b]), int(te[2][b + 1])
                    rows[s0:s1] = te[0][e0:e1] - b * brows
                    offs[s0:s1] = te[1][e0:e1]
                    ktag[s0:s1] = ki
                blk = rows.astype(np.int16).reshape(-1, 16).T  # [16, n/16]
                c0 = int(cb[si, b])
                idx_all[c, :, c0 * 8: c0 * 8 + nslot // 16] = np.tile(
                    blk, (8, 1))
                # off columns for builds of this bank
                for (ki, gcol, col, _s, _e) in builds[si]:
                    ch = gcol - int(gbase[si, b])
                    if not (0 <= ch < int(C[si, b])):
                        continue
                    sl = slice(ch * 128, (ch + 1) * 128)
                    off_all[c, :, col] = np.where(
                        ktag[sl] == ki, offs[sl], 512.0)

    return dict(sgs=sgs, C=C, gbase=gbase, builds=builds, n_cols=n_cols,
                idx=idx_all, off=off_all, nseg=total_chunks)


def preprocess(feats, W, b, prelu_a, src_pos, dst_pos, src_neg, dst_neg,
               cfg: Config):
    n, ncores, tc = cfg.n_nodes, cfg.n_cores, cfg.t_core
    feats = np.asarray(feats, np.float32)
    W = np.asarray(W, np.float32)
    b = np.asarray(b, np.float32)
    prelu_a = np.asarray(prelu_a, np.float32)

    xs, plans, layouts, nds = [], [], [], []
    for src, dst in ((src_pos, dst_pos), (src_neg, dst_neg)):
        src = np.asarray(src, np.int64)
        dst = np.asarray(dst, np.int64)
        dego = np.bincount(src, minlength=n).astype(np.float64)
        degi = np.bincount(dst, minlength=n).astype(np.float64)
        ns = np.where(dego > 0, 1.0 / np.sqrt(np.maximum(dego, 1.0)), 0.0)
        nd = np.where(degi > 0, 1.0 / np.sqrt(np.maximum(degi, 1.0)), 0.0)
        x = np.zeros((cfg.n_pad, D), np.float16)
        x[:n] = (feats * ns[:, None].astype(np.float32)).astype(np.float16)
        xs.append(x)
        nds.append(nd.astype(np.float32))
        cnt = np.bincount(dst // P, minlength=cfg.t_global)
        ct = _deal_tiles(cnt, cfg)
        plans.append(dict(core_tiles=ct))
        layouts.append(_layout_graph(src, dst, ct, cfg))

    # nd per (graph, position, core): [ncores, P, 2*t_core] f32
    nd_arr = np.zeros((ncores, P, 2 * tc), np.float32)
    for g in range(2):
        ndpad = np.zeros(cfg.n_pad, np.float32)
        ndpad[:n] = nds[g]
        ndt = ndpad.reshape(cfg.t_global, P).T
        ct = plans[g]["core_tiles"]
        for c in range(ncores):
            valid = ct[c] >= 0
            nd_arr[c][:, g * tc: (g + 1) * tc][:, valid] = ndt[:, ct[c][valid]]

    iota = np.tile(np.arange(P, dtype=np.float32), (P, 1)).astype(np.float16)
    a_rep = np.full((P, 1), float(prelu_a.reshape(-1)[0]), np.float32)
    b_rep = np.tile(b.reshape(1, D), (P, 1)).astype(np.float32)

    in_maps = []
    for c in range(ncores):
        in_maps.append({
            "x0": xs[0], "x1": xs[1],
            "w_in": W, "a_rep": a_rep, "b_rep": b_rep,
            "nd_in": nd_arr[c],
            "idx_in": np.concatenate(
                [layouts[0]["idx"][c], layouts[1]["idx"][c]], axis=1),
            "off_in": np.concatenate(
                [layouts[0]["off"][c], layouts[1]["off"][c]], axis=1),
            "iota_in": iota,
        })
    meta = {
        "layouts": layouts,
        "use_bias": bool(np.any(b != 0.0)),
    }
    return in_maps, plans, meta


# --------------------------------------------------------------------------
# Device kernel builder
# --------------------------------------------------------------------------
def build_kernel(nc, tc, cfg: Config, meta):
    from contextlib import ExitStack

    import concourse.mybir as mybir

    f32 = mybir.dt.float32
    f16 = mybir.dt.float16
    i16 = mybir.dt.int16
    Alu = mybir.AluOpType
    Act = mybir.ActivationFunctionType

    tcn, npad = cfg.t_core, cfg.n_pad
    layouts = meta["layouts"]
    use_bias = meta["use_bias"]
    nseg = [layouts[g]["nseg"] for g in range(2)]
    ncols = [layouts[g]["n_cols"] for g in range(2)]
    # max chunks/cols per supergroup (for fixed-size pool tiles)
    sg_chunks = []
    sg_cols = []
    for g in range(2):
        for si in range(len(layouts[g]["sgs"])):
            sg_chunks.append(int(layouts[g]["C"][si].sum()))
            sg_cols.append(len(layouts[g]["builds"][si]))
    cmax = max(sg_chunks)
    colmax = max(sg_cols)
    knmax = max(kn for g in range(2) for (_k0, kn) in layouts[g]["sgs"])

    x_dram = [nc.dram_tensor(f"x{g}", [npad, D], f16, kind="ExternalInput").ap()
              for g in range(2)]
    w_in = nc.dram_tensor("w_in", [P, D], f32, kind="ExternalInput").ap()
    a_rep = nc.dram_tensor("a_rep", [P, 1], f32, kind="ExternalInput").ap()
    b_rep = nc.dram_tensor("b_rep", [P, D], f32, kind="ExternalInput").ap()
    nd_in = nc.dram_tensor("nd_in", [P, 2 * tcn], f32, kind="ExternalInput").ap()
    idx_in = nc.dram_tensor("idx_in", [P, 8 * sum(nseg)], i16,
                            kind="ExternalInput").ap()
    off_in = nc.dram_tensor("off_in", [P, sum(ncols)], f32,
                            kind="ExternalInput").ap()
    iota_in = nc.dram_tensor("iota_in", [P, P], f16, kind="ExternalInput").ap()
    out = nc.dram_tensor("out", [2, P, tcn, D], f16, kind="ExternalOutput").ap()

    with ExitStack() as ctx:
        const = ctx.enter_context(tc.tile_pool(name="const", bufs=1))
        gpool = ctx.enter_context(tc.tile_pool(name="gpool", bufs=cfg.gbufs))
        ipool = ctx.enter_context(tc.tile_pool(name="ipool", bufs=2))
        opool = ctx.enter_context(tc.tile_pool(name="opool", bufs=2))
        ohpool = ctx.enter_context(tc.tile_pool(name="ohpool", bufs=8))
        apool = ctx.enter_context(tc.tile_pool(name="apool", bufs=3))
        spool = ctx.enter_context(tc.tile_pool(name="spool", bufs=2))
        tpool = ctx.enter_context(tc.tile_pool(name="tpool", bufs=4))
        ppool = ctx.enter_context(tc.tile_pool(name="ppool", bufs=4,
                                               space="PSUM"))
        hpool = ctx.enter_context(tc.tile_pool(name="hpool", bufs=3,
                                               space="PSUM"))

        # ---- constants ----
        w_sb = const.tile([P, D], f16)
        nc.gpsimd.dma_start(out=w_sb[:], in_=w_in)  # f32 -> f16 cast DMA
        iota_sb = const.tile([P, P], f16)
        nc.sync.dma_start(out=iota_sb[:], in_=iota_in)
        a_sb = const.tile([P, 1], f32)
        nc.sync.dma_start(out=a_sb[:], in_=a_rep)
        nd_sb = const.tile([P, 2 * tcn], f32)
        nc.sync.dma_start(out=nd_sb[:], in_=nd_in)
        if use_bias:
            b_sb = const.tile([P, D], f32)
            nc.sync.dma_start(out=b_sb[:], in_=b_rep)

        idx_base = [0, 8 * nseg[0]]
        col_base = [0, ncols[0]]
        # job list: interleave the two graphs' supergroups
        jobs = []
        for si in range(max(len(layouts[0]["sgs"]), len(layouts[1]["sgs"]))):
            for g in range(2):
                if si < len(layouts[g]["sgs"]):
                    jobs.append((g, si))

        # running chunk/col offsets per graph
        coff = [np.concatenate([[0], np.cumsum(
            layouts[g]["C"].reshape(-1))]).astype(int) for g in range(2)]
        boff = [np.concatenate([[0], np.cumsum(
            [len(bl) for bl in layouts[g]["builds"]])]).astype(int)
            for g in range(2)]

        obuild = 0  # global one-hot build counter for engine assignment
        for (g, si) in jobs:
            lay = layouts[g]
            (k0, kn) = lay["sgs"][si]
            Crow = lay["C"][si]
            nch = int(Crow.sum())
            c0 = int(coff[g][si * NBANK])   # first chunk of this sg
            bl = lay["builds"][si]
            col0 = int(boff[g][si])

            it = ipool.tile([P, cmax * 8], i16, tag="gidx")
            nc.sync.dma_start(
                out=it[:, : nch * 8],
                in_=idx_in[:, idx_base[g] + c0 * 8:
                           idx_base[g] + (c0 + nch) * 8])
            ot = opool.tile([P, colmax], f32, tag="goff")
            nc.sync.dma_start(
                out=ot[:, : len(bl)],
                in_=off_in[:, col_base[g] + col0:
                           col_base[g] + col0 + len(bl)])
            gt = gpool.tile([P, cmax, D], f16, tag="gather")
            for b in range(NBANK):
                Cb = int(Crow[b])
                if Cb == 0:
                    continue
                lo = int(lay["gbase"][si, b])
                rows = min(cfg.bank_rows, npad - b * cfg.bank_rows)
                nc.gpsimd.dma_gather(
                    out_ap=gt[:, lo: lo + Cb, :],
                    in_ap=x_dram[g][b * cfg.bank_rows:
                                    b * cfg.bank_rows + rows, :],
                    idxs_ap=it[:, lo * 8: (lo + Cb) * 8],
                    num_idxs=Cb * P, num_idxs_reg=Cb * P,
                    elem_size=D, single_packet=False)

            stg = spool.tile([P, knmax, D], f16, tag="stg")
            # group builds by tile position
            by_k = {}
            for (ki, gcol, col, s, e) in bl:
                by_k.setdefault(ki, []).append((gcol, col, s, e))
            for ki in range(kn):
                ps = ppool.tile([P, D], f32)
                for (gcol, col, s, e) in by_k[ki]:
                    oh = ohpool.tile([P, P], f16)
                    eng = nc.vector
                    if cfg.oh_mod and (obuild % cfg.oh_mod == 0):
                        eng = nc.gpsimd
                    obuild += 1
                    eng.tensor_scalar(
                        out=oh[:], in0=iota_sb[:],
                        scalar1=ot[:, col - col0: col - col0 + 1],
                        scalar2=None, op0=Alu.is_equal)
                    nc.tensor.matmul(out=ps[:], lhsT=gt[:, gcol, :],
                                     rhs=oh[:], start=s, stop=e)
                # aggT (psum, [d, j]) -> SBUF f16
                at = apool.tile([P, D], f16, tag="aggT")
                nc.scalar.activation(out=at[:], in_=ps[:], func=Act.Copy)
                hp = hpool.tile([P, D], f32)
                nc.tensor.matmul(out=hp[:], lhsT=at[:], rhs=w_sb[:],
                                 start=True, stop=True)
                kslot = g * tcn + (k0 + ki)
                if cfg.act_prelu and not use_bias:
                    nc.scalar.activation(
                        out=stg[:, ki, :], in_=hp[:], func=Act.Prelu,
                        scale=nd_sb[:, kslot: kslot + 1], alpha=a_sb[:, :1])
                else:
                    h1 = tpool.tile([P, D], f32, tag="h1")
                    nc.vector.tensor_scalar(
                        out=h1[:], in0=hp[:],
                        scalar1=nd_sb[:, kslot: kslot + 1],
                        scalar2=None, op0=Alu.mult)
                    if use_bias:
                        h2 = tpool.tile([P, D], f32, tag="h2")
                        nc.vector.tensor_tensor(out=h2[:], in0=h1[:],
                                                in1=b_sb[:], op=Alu.add)
                        h1 = h2
                    neg = tpool.tile([P, D], f32, tag="neg")
                    nc.vector.tensor_scalar(
                        out=neg[:], in0=h1[:], scalar1=0.0,
                        scalar2=a_sb[:, :1], op0=Alu.min, op1=Alu.mult)
                    pos = tpool.tile([P, D], f32, tag="pos")
                    nc.vector.tensor_scalar(
                        out=pos[:], in0=h1[:], scalar1=0.0,
                        scalar2=None, op0=Alu.max)
                    nc.vector.tensor_tensor(out=stg[:, ki, :], in0=neg[:],
                                            in1=pos[:], op=Alu.add)
            nc.sync.dma_start(out=out[g, :, k0: k0 + kn, :],
                              in_=stg[:, :kn, :])
    return out


# --------------------------------------------------------------------------
# Driver
# --------------------------------------------------------------------------
def _build_program(cfg: Config, meta):
    import concourse.bacc as bacc
    import concourse.tile as tile

    nc = bacc.Bacc("TRN2", target_bir_lowering=False, debug=False,
                   enable_asserts=False, num_devices=cfg.n_cores)
    with tile.TileContext(nc) as tc:
        build_kernel(nc, tc, cfg, meta)
    nc.compile()
    return nc


def _unscramble(results, plans, cfg: Config):
    n = cfg.n_nodes
    full = np.zeros((2, n, D), np.float32)
    for g in range(2):
        ct_all = plans[g]["core_tiles"]
        for core in range(cfg.n_cores):
            oc = np.asarray(results[core]["out"], np.float32)  # [2,P,tc,D]
            for k in range(cfg.t_core):
                t = int(ct_all[core, k])
                if t < 0:
                    continue
                r0 = t * P
                r1 = min(r0 + P, n)
                full[g, r0:r1] = oc[g, : r1 - r0, k, :]
    return full


_PROGRAM_CACHE = {}


def _meta_key(cfg: Config, meta):
    import hashlib
    hsh = hashlib.sha256()
    for g in range(2):
        lay = meta["layouts"][g]
        hsh.update(lay["C"].tobytes())
        for bl in lay["builds"]:
            hsh.update(np.asarray(bl, np.int64).tobytes())
    return (cfg.n_nodes, cfg.n_cores, cfg.sg, cfg.oh_mod, cfg.act_prelu,
            cfg.gbufs, meta["use_bias"], hsh.hexdigest())


def run(inputs, cfg: Config, trace=False):
    from concourse.bass_utils import run_bass_kernel_spmd

    in_maps, plans, meta = preprocess(
        inputs["feats"], inputs["W"], inputs["b"], inputs["prelu_a"],
        inputs["src_pos"], inputs["dst_pos"],
        inputs["src_neg"], inputs["dst_neg"], cfg)

    key = _meta_key(cfg, meta)
    nc = _PROGRAM_CACHE.get(key)
    if nc is None:
        nc = _build_program(cfg, meta)
        _PROGRAM_CACHE[key] = nc

    kwargs = {}
    if trace:
        kwargs = dict(trace=True, tmpdir=tempfile.mkdtemp(prefix="bgc_trace_"))
    res = run_bass_kernel_spmd(nc, in_maps, core_ids=list(range(cfg.n_cores)),
                               **kwargs)
    full = _unscramble(res.results, plans, cfg)
    return full, res


def kernel(**inputs) -> np.ndarray:
    cfg = Config()
    full, _ = run(inputs, cfg)
    return full


# revision 18
# speedup vs baseline: 2.2267x; 1.4152x over previous
"""Bass/Trainium2 kernel for BiGraphContrastLayer (GNN message passing).

Computes, for two edge lists (pos/neg) over the same node features:
    h_g = PReLU( D_in^-1/2 A_g D_out^-1/2 feats @ W + b )
returning stack([h_pos, h_neg]) of shape [2, N, Dout].

Strategy (8 NeuronCores, SPMD, no collectives). Using linearity,
    (D_in^-1/2 A D_out^-1/2 feats) @ W = D_in^-1/2 (A (D_out^-1/2 feats)) @ W
so the device aggregates raw (host-prescaled) feature rows FIRST and applies
W once per destination tile afterwards — there is no feats@W precompute
phase and no intermediate DRAM round trip at all:

  Host: x_g = f16(ns_g * feats)  (per-graph out-degree prescale), edges
  bucketed by dst tile, dst tiles dealt to cores (prefix-balanced so all 8
  cores share one instruction stream), edges packed into 128-slot chunks at
  supergroup x bank granularity (bank = 32K-row window for int16 gather
  indices; padding ~2%).

  Device, per (graph, supergroup) job:
    dma_gather pulls x[src] rows for each bank region into SBUF; per dst
    tile, one-hot matmuls (lhsT = gathered rows, rhs = is_equal(iota, off))
    segment-sum into a TRANSPOSED PSUM accumulator aggT[d, j]; ScalarE
    copies aggT to SBUF f16; one matmul aggT^T @ W -> h[j, d']; ScalarE
    PReLU with the in-degree norm nd folded into the activation scale
    (prelu(s*x) = s*prelu(x) for s>=0). f16 output, host upcasts.
"""

import math
import tempfile
from dataclasses import dataclass

import numpy as np

P = 128   # partitions
D = 128   # feature dim (Din == Dout == 128)
NBANK = 4
QUAD = 4  # dst-tile positions sharing one [P, 512] PSUM bank


# --------------------------------------------------------------------------
# Config
# --------------------------------------------------------------------------
@dataclass
class Config:
    n_nodes: int = 100000
    n_cores: int = 8
    sg: int = 20       # dst-tile positions per supergroup
    oh_mod: int = 3    # every oh_mod-th one-hot build goes to GpSimd (0=off)
    act_prelu: bool = True   # final PReLU on ScalarE (not in CoreSim)
    gbufs: int = 2           # gather buffer count

    @property
    def t_global(self) -> int:
        return math.ceil(self.n_nodes / P)

    @property
    def n_pad(self) -> int:
        return self.t_global * P

    @property
    def t_core(self) -> int:
        return math.ceil(self.t_global / self.n_cores)

    @property
    def bank_tiles(self) -> int:
        return math.ceil(self.t_global / NBANK)

    @property
    def bank_rows(self) -> int:
        return self.bank_tiles * P


# --------------------------------------------------------------------------
# Host-side preprocessing
# --------------------------------------------------------------------------
def _deal_tiles(bcnt, cfg: Config):
    """Deal tiles to cores with PER-BANK prefix balancing: sort tiles by
    total count desc; per group of n_cores, assign tiles (largest first) to
    the core whose per-bank running totals would deviate least from the
    group mean. Keeps each core's per-bank slot prefixes aligned so the
    shared (union) build structure has minimal slack.
    bcnt: [t_global, NBANK] per-tile per-bank edge counts.
    Returns core_tiles [n_cores, t_core] (-1 = null)."""
    nc, tc = cfg.n_cores, cfg.t_core
    tot = bcnt.sum(axis=1)
    order = np.argsort(-tot, kind="stable")
    core_tiles = np.full((nc, tc), -1, np.int64)
    cum = np.zeros((nc, NBANK), np.float64)
    for k in range(tc):
        grp = order[k * nc: (k + 1) * nc]
        taken = np.zeros(nc, bool)
        for t in grp:                       # biggest tile first
            best, bcost = -1, None
            newmean = (cum.sum(0) + bcnt[t]) / nc
            for c in range(nc):
                if taken[c]:
                    continue
                cost = float(((cum[c] + bcnt[t] - newmean) ** 2).sum())
                if bcost is None or cost < bcost:
                    best, bcost = c, cost
            taken[best] = True
            core_tiles[best, k] = t
            cum[best] += bcnt[t]
    return core_tiles


def _sg_split(tc, sg):
    """Split positions into supergroups of ~sg, with a tapered tail so the
    final jobs are small (shrinks the compute-only tail after the last
    gather)."""
    sizes = []
    rem = tc
    while rem > 2 * sg:
        sizes.append(sg)
        rem -= sg
    while rem > 4:
        piece = -(-rem // 2)
        sizes.append(piece)
        rem -= piece
    if rem:
        sizes.append(rem)
    out = []
    k0 = 0
    for s in sizes:
        out.append((k0, s))
        k0 += s
    return out


def _layout_graph(src, dst, core_tiles, cfg: Config):
    """Shared chunk/build layout for one graph + per-core idx/off data.

    Returns dict:
      sgs:    list of (k0, kn)
      C:      [n_sg, NBANK] shared chunk counts
      gbase:  [n_sg, NBANK] chunk offset of bank region within the sg tile
      builds: per sg, list of (ki, gcol, col, start, stop); col is global
              off-column index; gcol is chunk index within the sg gather tile
      n_cols: total off columns
      idx:    per-core [P, total_chunks*8] int16 (wrapped gather indices)
      off:    per-core [P, n_cols] f32
      nseg:   total chunks (sum of C)
    """
    ncores, tc = cfg.n_cores, cfg.t_core
    sgs = _sg_split(tc, cfg.sg)
    n_sg = len(sgs)
    brows = cfg.bank_rows

    # per-core, per-tile-position, per-bank edge lists (rows, offs)
    tile_edges = [[None] * tc for _ in range(ncores)]
    order = np.argsort(dst, kind="stable")
    src_s = src[order]
    dst_s = dst[order]
    tstart = np.zeros(cfg.t_global + 1, np.int64)
    np.cumsum(np.bincount(dst_s // P, minlength=cfg.t_global), out=tstart[1:])
    for c in range(ncores):
        for k in range(tc):
            t = core_tiles[c, k]
            if t < 0:
                tile_edges[c][k] = None
                continue
            e0, e1 = int(tstart[t]), int(tstart[t + 1])
            rows = src_s[e0:e1]
            offs = (dst_s[e0:e1] % P).astype(np.int64)
            bank = rows // brows
            bo = np.argsort(bank, kind="stable")
            rows, offs, bank = rows[bo], offs[bo], bank[bo]
            bcut = np.searchsorted(bank, np.arange(NBANK + 1))
            tile_edges[c][k] = (rows, offs, bcut)

    C = np.zeros((n_sg, NBANK), np.int64)
    gbase = np.zeros((n_sg, NBANK), np.int64)
    seg = []  # per sg, per bank: per core: list of (k, s0, s1) slot ranges
    for si, (k0, kn) in enumerate(sgs):
        for b in range(NBANK):
            percore = []
            maxm = 0
            for c in range(ncores):
                pos = 0
                rl = []
                for ki in range(kn):
                    te = tile_edges[c][k0 + ki]
                    if te is None:
                        rl.append((ki, pos, pos))
                        continue
                    n = int(te[2][b + 1] - te[2][b])
                    rl.append((ki, pos, pos + n))
                    pos += n
                percore.append(rl)
                maxm = max(maxm, pos)
            C[si, b] = -(-maxm // 128)
            seg.append(percore)
        C[si, 0] = max(C[si, 0], 1)  # dummy-build anchor
        gbase[si] = np.concatenate([[0], np.cumsum(C[si])[:-1]])

    # shared build list at QUAD granularity: 4 consecutive tile positions
    # share one [P, 512] PSUM bank; a build covers one chunk's intersection
    # with one quad, with a column window spanning the (cross-core union of)
    # tiles it touches. First/last build per quad use the full quad window
    # so accumulate start/stop flags are uniform.
    # build entry: (q0, gcol, col, wbase_tl, wtiles, start, stop)
    builds = []
    n_cols = 0
    for si, (k0, kn) in enumerate(sgs):
        bl = []
        for q0 in range(0, kn, QUAD):
            qn = min(QUAD, kn - q0)
            ent = []  # (b, ch, tl_min, tl_max)
            for b in range(NBANK):
                percore = seg[si * NBANK + b]
                # per chunk: union tile window
                win = {}
                for c in range(ncores):
                    for ki in range(q0, q0 + qn):
                        _, s0, s1 = percore[c][ki]
                        if s1 <= s0:
                            continue
                        for ch in range(s0 // 128, -(-s1 // 128)):
                            tl = ki - q0
                            if ch in win:
                                a, z = win[ch]
                                win[ch] = (min(a, tl), max(z, tl))
                            else:
                                win[ch] = (tl, tl)
                for ch in sorted(win):
                    a, z = win[ch]
                    ent.append((b, ch, a, z))
            if not ent:
                ent.append((0, 0, 0, qn - 1))  # dummy: zeroes the psum
            nb = len(ent)
            for j, (b, ch, a, z) in enumerate(ent):
                first = j == 0
                last = j == nb - 1
                if first or last:
                    a, z = 0, qn - 1  # full quad window
                bl.append((q0, int(gbase[si, b] + ch), n_cols, a, z - a + 1,
                           first, last))
                n_cols += 1
        builds.append(bl)

    # per-core arrays
    total_chunks = int(C.sum())
    idx_all = np.zeros((ncores, P, total_chunks * 8), np.int16)
    off_all = np.full((ncores, P, n_cols), 512.0, np.float32)
    # chunk column base per (si, b)
    cb = np.concatenate([[0], np.cumsum(C.reshape(-1))[:-1]]).reshape(
        n_sg, NBANK)
    for si, (k0, kn) in enumerate(sgs):
        for b in range(NBANK):
            nslot = int(C[si, b]) * 128
            if nslot == 0:
                continue
            percore = seg[si * NBANK + b]
            for c in range(ncores):
                rows = np.zeros(nslot, np.int64)
                offs = np.full(nslot, 512.0, np.float32)
                ktag = np.full(nslot, -1, np.int64)
                for (ki, s0, s1) in percore[c]:
                    if s1 == s0:
                        continue
                    te = tile_edges[c][k0 + ki]
                    e0, e1 = int(te[2][b]), int(te[2][b + 1])
                    rows[s0:s1] = te[0][e0:e1] - b * brows
                    offs[s0:s1] = te[1][e0:e1]
                    ktag[s0:s1] = ki
                blk = rows.astype(np.int16).reshape(-1, 16).T  # [16, n/16]
                c0 = int(cb[si, b])
                idx_all[c, :, c0 * 8: c0 * 8 + nslot // 16] = np.tile(
                    blk, (8, 1))
                # off columns for builds of this bank
                for (q0, gcol, col, a, w, _s, _e) in builds[si]:
                    ch = gcol - int(gbase[si, b])
                    if not (0 <= ch < int(C[si, b])):
                        continue
                    sl = slice(ch * 128, (ch + 1) * 128)
                    kt = ktag[sl]
                    inw = (kt >= q0 + a) & (kt < q0 + a + w)
                    off_all[c, :, col] = np.where(
                        inw, (kt - (q0 + a)) * 128 + offs[sl], 512.0)

    return dict(sgs=sgs, C=C, gbase=gbase, builds=builds, n_cols=n_cols,
                idx=idx_all, off=off_all, nseg=total_chunks)


def preprocess(feats, W, b, prelu_a, src_pos, dst_pos, src_neg, dst_neg,
               cfg: Config):
    n, ncores, tc = cfg.n_nodes, cfg.n_cores, cfg.t_core
    feats = np.asarray(feats, np.float32)
    W = np.asarray(W, np.float32)
    b = np.asarray(b, np.float32)
    prelu_a = np.asarray(prelu_a, np.float32)

    xs, plans, layouts, nds = [], [], [], []
    for src, dst in ((src_pos, dst_pos), (src_neg, dst_neg)):
        src = np.asarray(src, np.int64)
        dst = np.asarray(dst, np.int64)
        dego = np.bincount(src, minlength=n).astype(np.float64)
        degi = np.bincount(dst, minlength=n).astype(np.float64)
        ns = np.where(dego > 0, 1.0 / np.sqrt(np.maximum(dego, 1.0)), 0.0)
        nd = np.where(degi > 0, 1.0 / np.sqrt(np.maximum(degi, 1.0)), 0.0)
        x = np.zeros((cfg.n_pad, D), np.float16)
        x[:n] = (feats * ns[:, None].astype(np.float32)).astype(np.float16)
        xs.append(x)
        nds.append(nd.astype(np.float32))
        bank = src // cfg.bank_rows
        bcnt = np.zeros((cfg.t_global, NBANK), np.int64)
        np.add.at(bcnt, ((dst // P), bank), 1)
        ct = _deal_tiles(bcnt, cfg)
        plans.append(dict(core_tiles=ct))
        layouts.append(_layout_graph(src, dst, ct, cfg))

    # nd per (graph, position, core): [ncores, P, 2*t_core] f32
    nd_arr = np.zeros((ncores, P, 2 * tc), np.float32)
    for g in range(2):
        ndpad = np.zeros(cfg.n_pad, np.float32)
        ndpad[:n] = nds[g]
        ndt = ndpad.reshape(cfg.t_global, P).T
        ct = plans[g]["core_tiles"]
        for c in range(ncores):
            valid = ct[c] >= 0
            nd_arr[c][:, g * tc: (g + 1) * tc][:, valid] = ndt[:, ct[c][valid]]

    iota = np.tile(np.arange(QUAD * P, dtype=np.float32),
                   (P, 1)).astype(np.float16)
    a_rep = np.full((P, 1), float(prelu_a.reshape(-1)[0]), np.float32)
    b_rep = np.tile(b.reshape(1, D), (P, 1)).astype(np.float32)

    in_maps = []
    for c in range(ncores):
        in_maps.append({
            "x0": xs[0], "x1": xs[1],
            "w_in": W, "a_rep": a_rep, "b_rep": b_rep,
            "nd_in": nd_arr[c],
            "idx_in": np.concatenate(
                [layouts[0]["idx"][c], layouts[1]["idx"][c]], axis=1),
            "off_in": np.concatenate(
                [layouts[0]["off"][c], layouts[1]["off"][c]], axis=1),
            "iota_in": iota,
        })
    meta = {
        "layouts": layouts,
        "use_bias": bool(np.any(b != 0.0)),
    }
    return in_maps, plans, meta


# --------------------------------------------------------------------------
# Device kernel builder
# --------------------------------------------------------------------------
def build_kernel(nc, tc, cfg: Config, meta):
    from contextlib import ExitStack

    import concourse.mybir as mybir

    f32 = mybir.dt.float32
    f16 = mybir.dt.float16
    i16 = mybir.dt.int16
    Alu = mybir.AluOpType
    Act = mybir.ActivationFunctionType

    tcn, npad = cfg.t_core, cfg.n_pad
    layouts = meta["layouts"]
    use_bias = meta["use_bias"]
    nseg = [layouts[g]["nseg"] for g in range(2)]
    ncols = [layouts[g]["n_cols"] for g in range(2)]
    # max chunks/cols per supergroup (for fixed-size pool tiles)
    sg_chunks = []
    sg_cols = []
    for g in range(2):
        for si in range(len(layouts[g]["sgs"])):
            sg_chunks.append(int(layouts[g]["C"][si].sum()))
            sg_cols.append(len(layouts[g]["builds"][si]))
    cmax = max(sg_chunks)
    colmax = max(sg_cols)
    knmax = max(kn for g in range(2) for (_k0, kn) in layouts[g]["sgs"])

    x_dram = [nc.dram_tensor(f"x{g}", [npad, D], f16, kind="ExternalInput").ap()
              for g in range(2)]
    w_in = nc.dram_tensor("w_in", [P, D], f32, kind="ExternalInput").ap()
    a_rep = nc.dram_tensor("a_rep", [P, 1], f32, kind="ExternalInput").ap()
    b_rep = nc.dram_tensor("b_rep", [P, D], f32, kind="ExternalInput").ap()
    nd_in = nc.dram_tensor("nd_in", [P, 2 * tcn], f32, kind="ExternalInput").ap()
    idx_in = nc.dram_tensor("idx_in", [P, 8 * sum(nseg)], i16,
                            kind="ExternalInput").ap()
    off_in = nc.dram_tensor("off_in", [P, sum(ncols)], f32,
                            kind="ExternalInput").ap()
    iota_in = nc.dram_tensor("iota_in", [P, QUAD * P], f16,
                             kind="ExternalInput").ap()
    out = nc.dram_tensor("out", [2, P, tcn, D], f16, kind="ExternalOutput").ap()

    pb = dict(gpool=cfg.gbufs, ipool=2, opool=2, ohpool=16, apool=4,
              spool=2, tpool=4, ppool=4, hpool=3)
    pb.update(globals().get("POOL_BUFS") or {})

    with ExitStack() as ctx:
        const = ctx.enter_context(tc.tile_pool(name="const", bufs=1))
        gpool = ctx.enter_context(tc.tile_pool(name="gpool", bufs=pb["gpool"]))
        ipool = ctx.enter_context(tc.tile_pool(name="ipool", bufs=pb["ipool"]))
        opool = ctx.enter_context(tc.tile_pool(name="opool", bufs=pb["opool"]))
        ohpool = ctx.enter_context(tc.tile_pool(name="ohpool",
                                                bufs=pb["ohpool"]))
        apool = ctx.enter_context(tc.tile_pool(name="apool", bufs=pb["apool"]))
        spool = ctx.enter_context(tc.tile_pool(name="spool", bufs=pb["spool"]))
        tpool = ctx.enter_context(tc.tile_pool(name="tpool", bufs=pb["tpool"]))
        ppool = ctx.enter_context(tc.tile_pool(name="ppool", bufs=pb["ppool"],
                                               space="PSUM"))
        hpool = ctx.enter_context(tc.tile_pool(name="hpool", bufs=pb["hpool"],
                                               space="PSUM"))

        # ---- constants ----
        w_sb = const.tile([P, D], f16)
        nc.gpsimd.dma_start(out=w_sb[:], in_=w_in)  # f32 -> f16 cast DMA
        iota_sb = const.tile([P, QUAD * P], f16)
        nc.sync.dma_start(out=iota_sb[:], in_=iota_in)
        a_sb = const.tile([P, 1], f32)
        nc.sync.dma_start(out=a_sb[:], in_=a_rep)
        nd_sb = const.tile([P, 2 * tcn], f32)
        nc.sync.dma_start(out=nd_sb[:], in_=nd_in)
        if use_bias:
            b_sb = const.tile([P, D], f32)
            nc.sync.dma_start(out=b_sb[:], in_=b_rep)

        idx_base = [0, 8 * nseg[0]]
        col_base = [0, ncols[0]]
        # job list: interleave the two graphs' supergroups
        jobs = []
        for si in range(max(len(layouts[0]["sgs"]), len(layouts[1]["sgs"]))):
            for g in range(2):
                if si < len(layouts[g]["sgs"]):
                    jobs.append((g, si))

        # running chunk/col offsets per graph
        coff = [np.concatenate([[0], np.cumsum(
            layouts[g]["C"].reshape(-1))]).astype(int) for g in range(2)]
        boff = [np.concatenate([[0], np.cumsum(
            [len(bl) for bl in layouts[g]["builds"]])]).astype(int)
            for g in range(2)]

        obuild = 0  # global one-hot build counter for engine assignment
        for (g, si) in jobs:
            lay = layouts[g]
            (k0, kn) = lay["sgs"][si]
            Crow = lay["C"][si]
            nch = int(Crow.sum())
            c0 = int(coff[g][si * NBANK])   # first chunk of this sg
            bl = lay["builds"][si]
            col0 = int(boff[g][si])

            it = ipool.tile([P, cmax * 8], i16, tag="gidx")
            nc.sync.dma_start(
                out=it[:, : nch * 8],
                in_=idx_in[:, idx_base[g] + c0 * 8:
                           idx_base[g] + (c0 + nch) * 8])
            ot = opool.tile([P, colmax], f32, tag="goff")
            nc.sync.dma_start(
                out=ot[:, : len(bl)],
                in_=off_in[:, col_base[g] + col0:
                           col_base[g] + col0 + len(bl)])
            gt = gpool.tile([P, cmax, D], f16, tag="gather")
            for b in range(NBANK):
                Cb = int(Crow[b])
                if Cb == 0:
                    continue
                lo = int(lay["gbase"][si, b])
                rows = min(cfg.bank_rows, npad - b * cfg.bank_rows)
                nc.gpsimd.dma_gather(
                    out_ap=gt[:, lo: lo + Cb, :],
                    in_ap=x_dram[g][b * cfg.bank_rows:
                                    b * cfg.bank_rows + rows, :],
                    idxs_ap=it[:, lo * 8: (lo + Cb) * 8],
                    num_idxs=Cb * P, num_idxs_reg=Cb * P,
                    elem_size=D, single_packet=False)

            stg = spool.tile([P, knmax, D], f16, tag="stg")
            # group builds by quad
            by_q = {}
            for (q0, gcol, col, a, w, s, e) in bl:
                by_q.setdefault(q0, []).append((gcol, col, a, w, s, e))
            for q0 in sorted(by_q):
                qn = min(QUAD, kn - q0)
                ps = ppool.tile([P, QUAD * D], f32)
                for (gcol, col, a, w, s, e) in by_q[q0]:
                    oh = ohpool.tile([P, QUAD * P], f16)
                    eng = nc.vector
                    if w == 1 and cfg.oh_mod and (obuild % cfg.oh_mod == 0):
                        eng = nc.gpsimd
                    obuild += 1
                    eng.tensor_scalar(
                        out=oh[:, : w * P], in0=iota_sb[:, : w * P],
                        scalar1=ot[:, col - col0: col - col0 + 1],
                        scalar2=None, op0=Alu.is_equal)
                    nc.tensor.matmul(
                        out=ps[:, a * D: (a + w) * D],
                        lhsT=gt[:, gcol, :], rhs=oh[:, : w * P],
                        start=s, stop=e)
                # aggT (psum, [d, quad*j]) -> SBUF f16, one copy per quad
                at = apool.tile([P, QUAD * D], f16, tag="aggT")
                nc.scalar.activation(out=at[:, : qn * D],
                                     in_=ps[:, : qn * D], func=Act.Copy)
                for ki in range(q0, q0 + qn):
                    tl = ki - q0
                    hp = hpool.tile([P, D], f32)
                    nc.tensor.matmul(out=hp[:], lhsT=at[:, tl * D: (tl + 1) * D],
                                     rhs=w_sb[:], start=True, stop=True)
                    kslot = g * tcn + (k0 + ki)
                    if cfg.act_prelu and not use_bias:
                        nc.scalar.activation(
                            out=stg[:, ki, :], in_=hp[:], func=Act.Prelu,
                            scale=nd_sb[:, kslot: kslot + 1],
                            alpha=a_sb[:, :1])
                    else:
                        h1 = tpool.tile([P, D], f32, tag="h1")
                        nc.vector.tensor_scalar(
                            out=h1[:], in0=hp[:],
                            scalar1=nd_sb[:, kslot: kslot + 1],
                            scalar2=None, op0=Alu.mult)
                        if use_bias:
                            h2 = tpool.tile([P, D], f32, tag="h2")
                            nc.vector.tensor_tensor(out=h2[:], in0=h1[:],
                                                    in1=b_sb[:], op=Alu.add)
                            h1 = h2
                        neg = tpool.tile([P, D], f32, tag="neg")
                        nc.vector.tensor_scalar(
                            out=neg[:], in0=h1[:], scalar1=0.0,
                            scalar2=a_sb[:, :1], op0=Alu.min, op1=Alu.mult)
                        pos = tpool.tile([P, D], f32, tag="pos")
                        nc.vector.tensor_scalar(
                            out=pos[:], in0=h1[:], scalar1=0.0,
                            scalar2=None, op0=Alu.max)
                        nc.vector.tensor_tensor(out=stg[:, ki, :], in0=neg[:],
                                                in1=pos[:], op=Alu.add)
            nc.sync.dma_start(out=out[g, :, k0: k0 + kn, :],
                              in_=stg[:, :kn, :])
    return out


# --------------------------------------------------------------------------
# Driver
# --------------------------------------------------------------------------
def _build_program(cfg: Config, meta):
    import concourse.bacc as bacc
    import concourse.tile as tile

    nc = bacc.Bacc("TRN2", target_bir_lowering=False, debug=False,
                   enable_asserts=False, num_devices=cfg.n_cores)
    with tile.TileContext(nc) as tc:
        build_kernel(nc, tc, cfg, meta)
    nc.compile()
    return nc


def _unscramble(results, plans, cfg: Config):
    n = cfg.n_nodes
    full = np.zeros((2, n, D), np.float32)
    for g in range(2):
        ct_all = plans[g]["core_tiles"]
        for core in range(cfg.n_cores):
            oc = np.asarray(results[core]["out"], np.float32)  # [2,P,tc,D]
            for k in range(cfg.t_core):
                t = int(ct_all[core, k])
                if t < 0:
                    continue
                r0 = t * P
                r1 = min(r0 + P, n)
                full[g, r0:r1] = oc[g, : r1 - r0, k, :]
    return full


_PROGRAM_CACHE = {}


def _meta_key(cfg: Config, meta):
    import hashlib
    hsh = hashlib.sha256()
    for g in range(2):
        lay = meta["layouts"][g]
        hsh.update(lay["C"].tobytes())
        for bl in lay["builds"]:
            hsh.update(np.asarray(bl, np.int64).tobytes())
    return (cfg.n_nodes, cfg.n_cores, cfg.sg, cfg.oh_mod, cfg.act_prelu,
            cfg.gbufs, meta["use_bias"], hsh.hexdigest())


def run(inputs, cfg: Config, trace=False):
    from concourse.bass_utils import run_bass_kernel_spmd

    in_maps, plans, meta = preprocess(
        inputs["feats"], inputs["W"], inputs["b"], inputs["prelu_a"],
        inputs["src_pos"], inputs["dst_pos"],
        inputs["src_neg"], inputs["dst_neg"], cfg)

    key = _meta_key(cfg, meta)
    nc = _PROGRAM_CACHE.get(key)
    if nc is None:
        nc = _build_program(cfg, meta)
        _PROGRAM_CACHE[key] = nc

    kwargs = {}
    if trace:
        kwargs = dict(trace=True, tmpdir=tempfile.mkdtemp(prefix="bgc_trace_"))
    res = run_bass_kernel_spmd(nc, in_maps, core_ids=list(range(cfg.n_cores)),
                               **kwargs)
    full = _unscramble(res.results, plans, cfg)
    return full, res


def kernel(**inputs) -> np.ndarray:
    cfg = Config()
    full, _ = run(inputs, cfg)
    return full


# revision 25
# speedup vs baseline: 2.2579x; 1.0140x over previous
"""Bass/Trainium2 kernel for BiGraphContrastLayer (GNN message passing).

Computes, for two edge lists (pos/neg) over the same node features:
    h_g = PReLU( D_in^-1/2 A_g D_out^-1/2 feats @ W + b )
returning stack([h_pos, h_neg]) of shape [2, N, Dout].

Strategy (8 NeuronCores, SPMD, no collectives). Using linearity,
    (D_in^-1/2 A D_out^-1/2 feats) @ W = D_in^-1/2 (A (D_out^-1/2 feats)) @ W
so the device aggregates raw (host-prescaled) feature rows FIRST and applies
W once per destination tile afterwards — there is no feats@W precompute
phase and no intermediate DRAM round trip at all:

  Host: x_g = f16(ns_g * feats)  (per-graph out-degree prescale), edges
  bucketed by dst tile, dst tiles dealt to cores (prefix-balanced so all 8
  cores share one instruction stream), edges packed into 128-slot chunks at
  supergroup x bank granularity (bank = 32K-row window for int16 gather
  indices; padding ~2%).

  Device, per (graph, supergroup) job:
    dma_gather pulls x[src] rows for each bank region into SBUF; per dst
    tile, one-hot matmuls (lhsT = gathered rows, rhs = is_equal(iota, off))
    segment-sum into a TRANSPOSED PSUM accumulator aggT[d, j]; ScalarE
    copies aggT to SBUF f16; one matmul aggT^T @ W -> h[j, d']; ScalarE
    PReLU with the in-degree norm nd folded into the activation scale
    (prelu(s*x) = s*prelu(x) for s>=0). f16 output, host upcasts.
"""

import math
import tempfile
from dataclasses import dataclass

import numpy as np

P = 128   # partitions
D = 128   # feature dim (Din == Dout == 128)
NBANK = 4
QUAD = 4  # dst-tile positions sharing one [P, 512] PSUM bank


# --------------------------------------------------------------------------
# Config
# --------------------------------------------------------------------------
@dataclass
class Config:
    n_nodes: int = 100000
    n_cores: int = 8
    sg: int = 20       # dst-tile positions per supergroup
    oh_mod: int = 3    # every oh_mod-th one-hot build goes to GpSimd (0=off)
    act_prelu: bool = True   # final PReLU on ScalarE (not in CoreSim)
    idx16: bool = False      # 16-partition idx load (real HW needs all)
    gbufs: int = 2           # gather buffer count

    @property
    def t_global(self) -> int:
        return math.ceil(self.n_nodes / P)

    @property
    def n_pad(self) -> int:
        return self.t_global * P

    @property
    def t_core(self) -> int:
        return math.ceil(self.t_global / self.n_cores)

    @property
    def bank_tiles(self) -> int:
        return math.ceil(self.t_global / NBANK)

    @property
    def bank_rows(self) -> int:
        return self.bank_tiles * P


# --------------------------------------------------------------------------
# Host-side preprocessing
# --------------------------------------------------------------------------
def _deal_tiles(bcnt, cfg: Config):
    """Deal tiles to cores with PER-BANK prefix balancing: sort tiles by
    total count desc; per group of n_cores, assign tiles (largest first) to
    the core whose per-bank running totals would deviate least from the
    group mean. Keeps each core's per-bank slot prefixes aligned so the
    shared (union) build structure has minimal slack.
    bcnt: [t_global, NBANK] per-tile per-bank edge counts.
    Returns core_tiles [n_cores, t_core] (-1 = null)."""
    nc, tc = cfg.n_cores, cfg.t_core
    tot = bcnt.sum(axis=1)
    order = np.argsort(-tot, kind="stable")
    core_tiles = np.full((nc, tc), -1, np.int64)
    cum = np.zeros((nc, NBANK), np.float64)
    for k in range(tc):
        grp = order[k * nc: (k + 1) * nc]
        taken = np.zeros(nc, bool)
        for t in grp:                       # biggest tile first
            best, bcost = -1, None
            newmean = (cum.sum(0) + bcnt[t]) / nc
            for c in range(nc):
                if taken[c]:
                    continue
                cost = float(((cum[c] + bcnt[t] - newmean) ** 2).sum())
                if bcost is None or cost < bcost:
                    best, bcost = c, cost
            taken[best] = True
            core_tiles[best, k] = t
            cum[best] += bcnt[t]
    return core_tiles


def _sg_split(tc, sg):
    """Split positions into supergroups of ~sg, with a tapered tail so the
    final jobs are small (shrinks the compute-only tail after the last
    gather)."""
    sizes = []
    rem = tc
    while rem > 2 * sg:
        sizes.append(sg)
        rem -= sg
    while rem > 4:
        piece = -(-rem // 2)
        sizes.append(piece)
        rem -= piece
    if rem:
        sizes.append(rem)
    out = []
    k0 = 0
    for s in sizes:
        out.append((k0, s))
        k0 += s
    return out


def _layout_graph(src, dst, core_tiles, cfg: Config):
    """Shared chunk/build layout for one graph + per-core idx/off data.

    Returns dict:
      sgs:    list of (k0, kn)
      C:      [n_sg, NBANK] shared chunk counts
      gbase:  [n_sg, NBANK] chunk offset of bank region within the sg tile
      builds: per sg, list of (ki, gcol, col, start, stop); col is global
              off-column index; gcol is chunk index within the sg gather tile
      n_cols: total off columns
      idx:    per-core [P, total_chunks*8] int16 (wrapped gather indices)
      off:    per-core [P, n_cols] f32
      nseg:   total chunks (sum of C)
    """
    ncores, tc = cfg.n_cores, cfg.t_core
    sgs = _sg_split(tc, cfg.sg)
    n_sg = len(sgs)
    brows = cfg.bank_rows

    # per-core, per-tile-position, per-bank edge lists (rows, offs)
    tile_edges = [[None] * tc for _ in range(ncores)]
    order = np.argsort(dst, kind="stable")
    src_s = src[order]
    dst_s = dst[order]
    tstart = np.zeros(cfg.t_global + 1, np.int64)
    np.cumsum(np.bincount(dst_s // P, minlength=cfg.t_global), out=tstart[1:])
    for c in range(ncores):
        for k in range(tc):
            t = core_tiles[c, k]
            if t < 0:
                tile_edges[c][k] = None
                continue
            e0, e1 = int(tstart[t]), int(tstart[t + 1])
            rows = src_s[e0:e1]
            offs = (dst_s[e0:e1] % P).astype(np.int64)
            bank = rows // brows
            bo = np.argsort(bank, kind="stable")
            rows, offs, bank = rows[bo], offs[bo], bank[bo]
            bcut = np.searchsorted(bank, np.arange(NBANK + 1))
            tile_edges[c][k] = (rows, offs, bcut)

    C = np.zeros((n_sg, NBANK), np.int64)
    M = np.zeros((n_sg, NBANK), np.int64)   # true max slots (<= C*128)
    gbase = np.zeros((n_sg, NBANK), np.int64)
    seg = []  # per sg, per bank: per core: list of (k, s0, s1) slot ranges
    for si, (k0, kn) in enumerate(sgs):
        for b in range(NBANK):
            percore = []
            maxm = 0
            for c in range(ncores):
                pos = 0
                rl = []
                for ki in range(kn):
                    te = tile_edges[c][k0 + ki]
                    if te is None:
                        rl.append((ki, pos, pos))
                        continue
                    n = int(te[2][b + 1] - te[2][b])
                    rl.append((ki, pos, pos + n))
                    pos += n
                percore.append(rl)
                maxm = max(maxm, pos)
            C[si, b] = -(-maxm // 128)
            M[si, b] = maxm
            seg.append(percore)
        C[si, 0] = max(C[si, 0], 1)  # dummy-build anchor
        M[si, 0] = max(M[si, 0], 1)
        gbase[si] = np.concatenate([[0], np.cumsum(C[si])[:-1]])

    # shared build list at QUAD granularity: 4 consecutive tile positions
    # share one [P, 512] PSUM bank; a build covers one chunk's intersection
    # with one quad, with a column window spanning the (cross-core union of)
    # tiles it touches. First/last build per quad use the full quad window
    # so accumulate start/stop flags are uniform.
    # build entry: (q0, gcol, col, wbase_tl, wtiles, start, stop)
    builds = []
    n_cols = 0
    for si, (k0, kn) in enumerate(sgs):
        bl = []
        for q0 in range(0, kn, QUAD):
            qn = min(QUAD, kn - q0)
            ent = []  # (b, ch, tl_min, tl_max)
            for b in range(NBANK):
                percore = seg[si * NBANK + b]
                # per chunk: union tile window
                win = {}
                for c in range(ncores):
                    for ki in range(q0, q0 + qn):
                        _, s0, s1 = percore[c][ki]
                        if s1 <= s0:
                            continue
                        for ch in range(s0 // 128, -(-s1 // 128)):
                            tl = ki - q0
                            if ch in win:
                                a, z = win[ch]
                                win[ch] = (min(a, tl), max(z, tl))
                            else:
                                win[ch] = (tl, tl)
                for ch in sorted(win):
                    a, z = win[ch]
                    ent.append((b, ch, a, z))
            if not ent:
                ent.append((0, 0, 0, qn - 1))  # dummy: zeroes the psum
            nb = len(ent)
            for j, (b, ch, a, z) in enumerate(ent):
                first = j == 0
                last = j == nb - 1
                if first or last:
                    a, z = 0, qn - 1  # full quad window
                bl.append((q0, int(gbase[si, b] + ch), n_cols, a, z - a + 1,
                           first, last))
                n_cols += 1
        builds.append(bl)

    # per-core arrays
    total_chunks = int(C.sum())
    idx_all = np.zeros((ncores, P, total_chunks * 8), np.int16)
    off_all = np.full((ncores, P, n_cols), 512.0, np.float32)
    # chunk column base per (si, b)
    cb = np.concatenate([[0], np.cumsum(C.reshape(-1))[:-1]]).reshape(
        n_sg, NBANK)
    for si, (k0, kn) in enumerate(sgs):
        for b in range(NBANK):
            nslot = int(C[si, b]) * 128
            if nslot == 0:
                continue
            percore = seg[si * NBANK + b]
            for c in range(ncores):
                rows = np.zeros(nslot, np.int64)
                offs = np.full(nslot, 512.0, np.float32)
                ktag = np.full(nslot, -1, np.int64)
                for (ki, s0, s1) in percore[c]:
                    if s1 == s0:
                        continue
                    te = tile_edges[c][k0 + ki]
                    e0, e1 = int(te[2][b]), int(te[2][b + 1])
                    rows[s0:s1] = te[0][e0:e1] - b * brows
                    offs[s0:s1] = te[1][e0:e1]
                    ktag[s0:s1] = ki
                blk = rows.astype(np.int16).reshape(-1, 16).T  # [16, n/16]
                c0 = int(cb[si, b])
                idx_all[c, :, c0 * 8: c0 * 8 + nslot // 16] = np.tile(
                    blk, (8, 1))
                # off columns for builds of this bank
                for (q0, gcol, col, a, w, _s, _e) in builds[si]:
                    ch = gcol - int(gbase[si, b])
                    if not (0 <= ch < int(C[si, b])):
                        continue
                    sl = slice(ch * 128, (ch + 1) * 128)
                    kt = ktag[sl]
                    inw = (kt >= q0 + a) & (kt < q0 + a + w)
                    off_all[c, :, col] = np.where(
                        inw, (kt - (q0 + a)) * 128 + offs[sl], 512.0)

    return dict(sgs=sgs, C=C, M=M, gbase=gbase, builds=builds, n_cols=n_cols,
                idx=idx_all, off=off_all, nseg=total_chunks)


def preprocess(feats, W, b, prelu_a, src_pos, dst_pos, src_neg, dst_neg,
               cfg: Config):
    n, ncores, tc = cfg.n_nodes, cfg.n_cores, cfg.t_core
    feats = np.asarray(feats, np.float32)
    W = np.asarray(W, np.float32)
    b = np.asarray(b, np.float32)
    prelu_a = np.asarray(prelu_a, np.float32)

    xs, plans, layouts, nds = [], [], [], []
    for src, dst in ((src_pos, dst_pos), (src_neg, dst_neg)):
        src = np.asarray(src, np.int64)
        dst = np.asarray(dst, np.int64)
        dego = np.bincount(src, minlength=n).astype(np.float64)
        degi = np.bincount(dst, minlength=n).astype(np.float64)
        ns = np.where(dego > 0, 1.0 / np.sqrt(np.maximum(dego, 1.0)), 0.0)
        nd = np.where(degi > 0, 1.0 / np.sqrt(np.maximum(degi, 1.0)), 0.0)
        x = np.zeros((cfg.n_pad, D), np.float16)
        x[:n] = (feats * ns[:, None].astype(np.float32)).astype(np.float16)
        xs.append(x)
        nds.append(nd.astype(np.float32))
        bank = src // cfg.bank_rows
        bcnt = np.zeros((cfg.t_global, NBANK), np.int64)
        np.add.at(bcnt, ((dst // P), bank), 1)
        ct = _deal_tiles(bcnt, cfg)
        plans.append(dict(core_tiles=ct))
        layouts.append(_layout_graph(src, dst, ct, cfg))

    # nd per (graph, position, core): [ncores, P, 2*t_core] f32
    nd_arr = np.zeros((ncores, P, 2 * tc), np.float32)
    for g in range(2):
        ndpad = np.zeros(cfg.n_pad, np.float32)
        ndpad[:n] = nds[g]
        ndt = ndpad.reshape(cfg.t_global, P).T
        ct = plans[g]["core_tiles"]
        for c in range(ncores):
            valid = ct[c] >= 0
            nd_arr[c][:, g * tc: (g + 1) * tc][:, valid] = ndt[:, ct[c][valid]]

    iota = np.tile(np.arange(QUAD * P, dtype=np.float32),
                   (P, 1)).astype(np.float16)
    a_rep = np.full((P, 1), float(prelu_a.reshape(-1)[0]), np.float32)
    b_rep = np.tile(b.reshape(1, D), (P, 1)).astype(np.float32)

    in_maps = []
    for c in range(ncores):
        in_maps.append({
            "x0": xs[0], "x1": xs[1],
            "w_in": W, "a_rep": a_rep, "b_rep": b_rep,
            "nd_in": nd_arr[c],
            "idx_in": np.concatenate(
                [layouts[0]["idx"][c], layouts[1]["idx"][c]], axis=1),
            "off_in": np.concatenate(
                [layouts[0]["off"][c], layouts[1]["off"][c]], axis=1),
            "iota_in": iota,
        })
    meta = {
        "layouts": layouts,
        "use_bias": bool(np.any(b != 0.0)),
    }
    return in_maps, plans, meta


# --------------------------------------------------------------------------
# Device kernel builder
# --------------------------------------------------------------------------
def build_kernel(nc, tc, cfg: Config, meta):
    from contextlib import ExitStack

    import concourse.mybir as mybir

    f32 = mybir.dt.float32
    f16 = mybir.dt.float16
    i16 = mybir.dt.int16
    Alu = mybir.AluOpType
    Act = mybir.ActivationFunctionType

    tcn, npad = cfg.t_core, cfg.n_pad
    layouts = meta["layouts"]
    use_bias = meta["use_bias"]
    nseg = [layouts[g]["nseg"] for g in range(2)]
    ncols = [layouts[g]["n_cols"] for g in range(2)]
    # max chunks/cols per supergroup (for fixed-size pool tiles)
    sg_chunks = []
    sg_cols = []
    for g in range(2):
        for si in range(len(layouts[g]["sgs"])):
            sg_chunks.append(int(layouts[g]["C"][si].sum()))
            sg_cols.append(len(layouts[g]["builds"][si]))
    cmax = max(sg_chunks)
    colmax = max(sg_cols)
    knmax = max(kn for g in range(2) for (_k0, kn) in layouts[g]["sgs"])

    x_dram = [nc.dram_tensor(f"x{g}", [npad, D], f16, kind="ExternalInput").ap()
              for g in range(2)]
    w_in = nc.dram_tensor("w_in", [P, D], f32, kind="ExternalInput").ap()
    a_rep = nc.dram_tensor("a_rep", [P, 1], f32, kind="ExternalInput").ap()
    b_rep = nc.dram_tensor("b_rep", [P, D], f32, kind="ExternalInput").ap()
    nd_in = nc.dram_tensor("nd_in", [P, 2 * tcn], f32, kind="ExternalInput").ap()
    idx_in = nc.dram_tensor("idx_in", [P, 8 * sum(nseg)], i16,
                            kind="ExternalInput").ap()
    off_in = nc.dram_tensor("off_in", [P, sum(ncols)], f32,
                            kind="ExternalInput").ap()
    iota_in = nc.dram_tensor("iota_in", [P, QUAD * P], f16,
                             kind="ExternalInput").ap()
    out = nc.dram_tensor("out", [2, P, tcn, D], f16, kind="ExternalOutput").ap()

    pb = dict(gpool=cfg.gbufs, ipool=2, opool=2, ohpool=16, apool=4,
              spool=2, tpool=4, ppool=4, hpool=3)
    pb.update(globals().get("POOL_BUFS") or {})

    with ExitStack() as ctx:
        const = ctx.enter_context(tc.tile_pool(name="const", bufs=1))
        gpool = ctx.enter_context(tc.tile_pool(name="gpool", bufs=pb["gpool"]))
        ipool = ctx.enter_context(tc.tile_pool(name="ipool", bufs=pb["ipool"]))
        opool = ctx.enter_context(tc.tile_pool(name="opool", bufs=pb["opool"]))
        ohpool = ctx.enter_context(tc.tile_pool(name="ohpool",
                                                bufs=pb["ohpool"]))
        apool = ctx.enter_context(tc.tile_pool(name="apool", bufs=pb["apool"]))
        spool = ctx.enter_context(tc.tile_pool(name="spool", bufs=pb["spool"]))
        tpool = ctx.enter_context(tc.tile_pool(name="tpool", bufs=pb["tpool"]))
        ppool = ctx.enter_context(tc.tile_pool(name="ppool", bufs=pb["ppool"],
                                               space="PSUM"))
        hpool = ctx.enter_context(tc.tile_pool(name="hpool", bufs=pb["hpool"],
                                               space="PSUM"))

        idx_base = [0, 8 * nseg[0]]
        col_base = [0, ncols[0]]
        # job list: interleave the two graphs' supergroups
        jobs = []
        for si in range(max(len(layouts[0]["sgs"]), len(layouts[1]["sgs"]))):
            for g in range(2):
                if si < len(layouts[g]["sgs"]):
                    jobs.append((g, si))

        # running chunk/col offsets per graph
        coff = [np.concatenate([[0], np.cumsum(
            layouts[g]["C"].reshape(-1))]).astype(int) for g in range(2)]
        boff = [np.concatenate([[0], np.cumsum(
            [len(bl) for bl in layouts[g]["builds"]])]).astype(int)
            for g in range(2)]
        # slot coverage written by each gather buffer's first-use job; later
        # jobs on the same buffer may use exact (unpadded) num_idxs only
        # where their region was fully pre-written (masked stale slots must
        # hold finite f16 data, not uninitialized SBUF).
        buf_cover = [0] * pb["gpool"]

        ipart = 16 if cfg.idx16 else P

        def issue_loads(jidx, g, si):
            lay = layouts[g]
            Crow = lay["C"][si]
            nch = int(Crow.sum())
            c0 = int(coff[g][si * NBANK])
            bl = lay["builds"][si]
            col0 = int(boff[g][si])
            it = ipool.tile([P, cmax * 8], i16, tag="gidx")
            nc.sync.dma_start(
                out=it[:ipart, : nch * 8],
                in_=idx_in[:ipart, idx_base[g] + c0 * 8:
                           idx_base[g] + (c0 + nch) * 8])
            ot = opool.tile([P, colmax], f32, tag="goff")
            nc.sync.dma_start(
                out=ot[:, : len(bl)],
                in_=off_in[:, col_base[g] + col0:
                           col_base[g] + col0 + len(bl)])
            gt = gpool.tile([P, cmax, D], f16, tag="gather")
            first_use = jidx < pb["gpool"]
            exact_ok = (not first_use) and nch <= buf_cover[jidx % pb["gpool"]]
            if first_use:
                buf_cover[jidx % pb["gpool"]] = nch
            for b in range(NBANK):
                Cb = int(Crow[b])
                if Cb == 0:
                    continue
                lo = int(lay["gbase"][si, b])
                rows = min(cfg.bank_rows, npad - b * cfg.bank_rows)
                ni = int(lay["M"][si, b]) if exact_ok else Cb * P
                icols = -(-ni // 16)
                nc.gpsimd.dma_gather(
                    out_ap=gt[:, lo: lo + Cb, :],
                    in_ap=x_dram[g][b * cfg.bank_rows:
                                    b * cfg.bank_rows + rows, :],
                    idxs_ap=it[:, lo * 8: lo * 8 + icols],
                    num_idxs=ni, num_idxs_reg=ni,
                    elem_size=D, single_packet=False)
            return it, ot, gt

        if cfg.idx16:
            # idx DMAs only write partitions 0..15 (all the gather reads);
            # zero the idx buffers once so the rest is initialized.
            for _ in range(pb["ipool"]):
                zt = ipool.tile([P, cmax * 8], i16, tag="gidx")
                nc.vector.memset(zt[:], 0)

        # job 0's loads + gathers go first so the DMA stream starts
        # immediately; constants follow (first needed ~10us in).
        pre = {0: issue_loads(0, *jobs[0])}

        # ---- constants ----
        iota_sb = const.tile([P, QUAD * P], f16)
        nc.sync.dma_start(out=iota_sb[:], in_=iota_in)
        w_sb = const.tile([P, D], f16)
        nc.gpsimd.dma_start(out=w_sb[:], in_=w_in)  # f32 -> f16 cast DMA
        a_sb = const.tile([P, 1], f32)
        nc.sync.dma_start(out=a_sb[:], in_=a_rep)
        nd_sb = const.tile([P, 2 * tcn], f32)
        nc.sync.dma_start(out=nd_sb[:], in_=nd_in)
        if use_bias:
            b_sb = const.tile([P, D], f32)
            nc.sync.dma_start(out=b_sb[:], in_=b_rep)

        obuild = 0  # global one-hot build counter for engine assignment
        for jidx, (g, si) in enumerate(jobs):
            lay = layouts[g]
            (k0, kn) = lay["sgs"][si]
            bl = lay["builds"][si]
            col0 = int(boff[g][si])
            it, ot, gt = pre.pop(jidx, None) or issue_loads(jidx, g, si)

            stg = spool.tile([P, knmax, D], f16, tag="stg")
            # group builds by quad
            by_q = {}
            for (q0, gcol, col, a, w, s, e) in bl:
                by_q.setdefault(q0, []).append((gcol, col, a, w, s, e))
            for q0 in sorted(by_q):
                qn = min(QUAD, kn - q0)
                ps = ppool.tile([P, QUAD * D], f32)
                for (gcol, col, a, w, s, e) in by_q[q0]:
                    oh = ohpool.tile([P, QUAD * P], f16)
                    eng = nc.vector
                    if w == 1 and cfg.oh_mod and (obuild % cfg.oh_mod == 0):
                        eng = nc.gpsimd
                    obuild += 1
                    eng.tensor_scalar(
                        out=oh[:, : w * P], in0=iota_sb[:, : w * P],
                        scalar1=ot[:, col - col0: col - col0 + 1],
                        scalar2=None, op0=Alu.is_equal)
                    nc.tensor.matmul(
                        out=ps[:, a * D: (a + w) * D],
                        lhsT=gt[:, gcol, :], rhs=oh[:, : w * P],
                        start=s, stop=e)
                # aggT (psum, [d, quad*j]) -> SBUF f16, one copy per quad
                at = apool.tile([P, QUAD * D], f16, tag="aggT")
                nc.scalar.activation(out=at[:, : qn * D],
                                     in_=ps[:, : qn * D], func=Act.Copy)
                for ki in range(q0, q0 + qn):
                    tl = ki - q0
                    hp = hpool.tile([P, D], f32)
                    nc.tensor.matmul(out=hp[:], lhsT=at[:, tl * D: (tl + 1) * D],
                                     rhs=w_sb[:], start=True, stop=True)
                    kslot = g * tcn + (k0 + ki)
                    if cfg.act_prelu and not use_bias:
                        nc.scalar.activation(
                            out=stg[:, ki, :], in_=hp[:], func=Act.Prelu,
                            scale=nd_sb[:, kslot: kslot + 1],
                            alpha=a_sb[:, :1])
                    else:
                        h1 = tpool.tile([P, D], f32, tag="h1")
                        nc.vector.tensor_scalar(
                            out=h1[:], in0=hp[:],
                            scalar1=nd_sb[:, kslot: kslot + 1],
                            scalar2=None, op0=Alu.mult)
                        if use_bias:
                            h2 = tpool.tile([P, D], f32, tag="h2")
                            nc.vector.tensor_tensor(out=h2[:], in0=h1[:],
                                                    in1=b_sb[:], op=Alu.add)
                            h1 = h2
                        neg = tpool.tile([P, D], f32, tag="neg")
                        nc.vector.tensor_scalar(
                            out=neg[:], in0=h1[:], scalar1=0.0,
                            scalar2=a_sb[:, :1], op0=Alu.min, op1=Alu.mult)
                        pos = tpool.tile([P, D], f32, tag="pos")
                        nc.vector.tensor_scalar(
                            out=pos[:], in0=h1[:], scalar1=0.0,
                            scalar2=None, op0=Alu.max)
                        nc.vector.tensor_tensor(out=stg[:, ki, :], in0=neg[:],
                                                in1=pos[:], op=Alu.add)
            nc.sync.dma_start(out=out[g, :, k0: k0 + kn, :],
                              in_=stg[:, :kn, :])
    return out


# --------------------------------------------------------------------------
# Driver
# --------------------------------------------------------------------------
def _build_program(cfg: Config, meta):
    import concourse.bacc as bacc
    import concourse.tile as tile

    nc = bacc.Bacc("TRN2", target_bir_lowering=False, debug=False,
                   enable_asserts=False, num_devices=cfg.n_cores)
    with tile.TileContext(nc) as tc:
        build_kernel(nc, tc, cfg, meta)
    nc.compile()
    return nc


def _unscramble(results, plans, cfg: Config):
    n = cfg.n_nodes
    full = np.zeros((2, n, D), np.float32)
    for g in range(2):
        ct_all = plans[g]["core_tiles"]
        for core in range(cfg.n_cores):
            oc = np.asarray(results[core]["out"], np.float32)  # [2,P,tc,D]
            for k in range(cfg.t_core):
                t = int(ct_all[core, k])
                if t < 0:
                    continue
                r0 = t * P
                r1 = min(r0 + P, n)
                full[g, r0:r1] = oc[g, : r1 - r0, k, :]
    return full


_PROGRAM_CACHE = {}


def _meta_key(cfg: Config, meta):
    import hashlib
    hsh = hashlib.sha256()
    for g in range(2):
        lay = meta["layouts"][g]
        hsh.update(lay["C"].tobytes())
        for bl in lay["builds"]:
            hsh.update(np.asarray(bl, np.int64).tobytes())
    return (cfg.n_nodes, cfg.n_cores, cfg.sg, cfg.oh_mod, cfg.act_prelu,
            cfg.gbufs, cfg.idx16, meta["use_bias"], hsh.hexdigest())


def run(inputs, cfg: Config, trace=False):
    from concourse.bass_utils import run_bass_kernel_spmd

    in_maps, plans, meta = preprocess(
        inputs["feats"], inputs["W"], inputs["b"], inputs["prelu_a"],
        inputs["src_pos"], inputs["dst_pos"],
        inputs["src_neg"], inputs["dst_neg"], cfg)

    key = _meta_key(cfg, meta)
    nc = _PROGRAM_CACHE.get(key)
    if nc is None:
        nc = _build_program(cfg, meta)
        _PROGRAM_CACHE[key] = nc

    kwargs = {}
    if trace:
        kwargs = dict(trace=True, tmpdir=tempfile.mkdtemp(prefix="bgc_trace_"))
    res = run_bass_kernel_spmd(nc, in_maps, core_ids=list(range(cfg.n_cores)),
                               **kwargs)
    full = _unscramble(res.results, plans, cfg)
    return full, res


def kernel(**inputs) -> np.ndarray:
    cfg = Config()
    full, _ = run(inputs, cfg)
    return full
